# revision 5
# baseline (speedup 1.0000x reference)
"""Trainium2 Bass kernel for nn_DiWeightedGCNLayer (8-core SPMD), v3.

Math (per reference):
    h   = (x - mu) * rsqrt(var + eps)            # LN, gamma folded into W2
    m   = (h * gamma) @ W.T + b = h @ W2 (+ c)
    out = segsum(m[src] * w, dst) / max(deg, 1) * dst_scale
    y   = x + gelu(out)

Key structure (v3):
  - Matmul associativity: sum_e w_e * (h[src_e] @ W2) = (sum_e w_e *
    onehot_e (x) h[src_e]) @ W2.  Phase 1 only computes/stores h (LN) in
    bf16; phase 2 gathers h rows, accumulates U^T = sum_b msg_b^T @ oh_b
    per 128-dst chunk in PSUM, then applies W2 once per chunk.
  - Gathers use batched InstDMAGatherAnt (dma_gather): one instruction per
    (group-of-chunks, src-region) instead of one indirect DMA per 128
    edges (SWDGE fixed cost ~1us each was the old bottleneck).
  - dma_gather indices are int16: h is stored in two region tensors with
    tiled layout hq[p, t, :] = h[t*128 + p], so the flat row index
    (s%128)*ntile + s//128 stays < 32768 for both regions.
  - x input is pre-tiled/bf16 on the host (xq[p, t, :] = x[t*128+p]) so
    every load/store is 2KB-contiguous per partition.
  - deg never computed on device: host folds dst_scale/max(deg,1) into a
    per-node scalar.
Engine balance: DVE bn_stats/bn_aggr + one-hots; Pool normalize +
gathers; ACT rsqrt + h-stores + U copies + gelu; PE scatter matmuls; SP
x/meta/xres/y DMAs.
"""

import contextlib
import numpy as np
import ml_dtypes

import concourse.bass as bass
import concourse.bacc as bacc
import concourse.tile as tile
import concourse.mybir as mybir
from concourse.bass_utils import run_bass_kernel_spmd

F32 = mybir.dt.float32
BF16 = mybir.dt.bfloat16
I16 = mybir.dt.int16
AF = mybir.ActivationFunctionType
OP = mybir.AluOpType

D = 128
P = 128
LN_EPS = 1e-5
REG0_TILES = 256          # LO region = nodes [0, 256*128) = [0, 32768)
GT = 8                    # phase-1 tiles per x-load/h-store group


def build_program(nt, bl, bh, groups, include_c=False, af_gelu=True,
                  loop_n=1, gq=4, debug_phase=0):
    """One-core SPMD program.

    nt: total 128-row node tiles (incl. padding).
    bl/bh: per chunk-index LO/HI block counts (tuples, len nch).
    groups: tuple of (chunk_lo, chunk_hi) chunk-index ranges per gather
        group.
    """
    nch = len(bl)
    ntL = REG0_TILES
    ntH = nt - REG0_TILES
    btot = sum(bl) + sum(bh)
    # per-chunk column offset into rel/w arrays
    coff = np.concatenate([[0], np.cumsum(np.array(bl) + np.array(bh))])
    # per-group idx column offsets (int16 cols = n_idx/16 = blocks*8)
    gNL = [sum(bl[a:b]) for a, b in groups]
    gNH = [sum(bh[a:b]) for a, b in groups]
    ioffL = np.concatenate([[0], np.cumsum([n * 8 for n in gNL])])
    ioffH = np.concatenate([[0], np.cumsum([n * 8 for n in gNH])])

    nc = bacc.Bacc(num_swdge_queues=gq)

    xq_ext = nc.declare_dram_parameter("xq", [P, nt, D], BF16, isOutput=False)
    xres_ext = nc.declare_dram_parameter("xres", [nch * P, D], BF16,
                                         isOutput=False)
    w2_ext = nc.declare_dram_parameter("w2", [D, D], BF16, isOutput=False)
    iota_ext = nc.declare_dram_parameter("iota", [P, P], BF16, isOutput=False)
    rel_ext = nc.declare_dram_parameter("rel", [P, btot], F32, isOutput=False)
    wgt_ext = nc.declare_dram_parameter("wgt", [P, btot], F32, isOutput=False)
    il_ext = nc.declare_dram_parameter("idxlo", [P, max(int(ioffL[-1]), 1)],
                                       I16, isOutput=False)
    ih_ext = nc.declare_dram_parameter("idxhi", [P, max(int(ioffH[-1]), 1)],
                                       I16, isOutput=False)
    ids_ext = nc.declare_dram_parameter("invdsct", [P, nch], F32,
                                        isOutput=False)
    if include_c:
        cb_ext = nc.declare_dram_parameter("cb", [P, D], F32, isOutput=False)
        qs_ext = nc.declare_dram_parameter("qsct", [P, nch], F32,
                                           isOutput=False)
    y_ext = nc.declare_dram_parameter("y", [nch * P, D], BF16, isOutput=True)

    hlo_dram = nc.dram_tensor("h_lo", [P * ntL, D], BF16)
    hhi_dram = nc.dram_tensor("h_hi", [P * ntH, D], BF16)

    with tile.TileContext(nc) as tc:
        with (
            tc.tile_pool(name="const", bufs=1) as const,
            tc.tile_pool(name="xp", bufs=4) as xp,
            tc.tile_pool(name="hp", bufs=4) as hp,
            tc.tile_pool(name="stats", bufs=5) as sp,
            tc.tile_pool(name="small", bufs=8) as smp,
            tc.tile_pool(name="msgL", bufs=2) as msgLp,
            tc.tile_pool(name="msgH", bufs=2) as msgHp,
            tc.tile_pool(name="meta", bufs=3) as metp,
            tc.tile_pool(name="oh", bufs=130) as ohp,
            tc.tile_pool(name="ut", bufs=4) as utp,
            tc.tile_pool(name="ep", bufs=8) as epp,
            tc.tile_pool(name="ps_u", bufs=6, space="PSUM") as ps_u,
            tc.tile_pool(name="ps_o", bufs=2, space="PSUM") as ps_o,
        ):
            # --- constants ---
            w2_t = const.tile([D, D], BF16)
            nc.sync.dma_start(out=w2_t[:], in_=w2_ext[:, :])
            iota_t = const.tile([P, P], BF16)
            nc.sync.dma_start(out=iota_t[:], in_=iota_ext[:, :])
            eps_t = const.tile([P, 1], F32)
            nc.vector.memset(eps_t[:], LN_EPS)
            ids_t = const.tile([P, nch], F32)
            nc.sync.dma_start(out=ids_t[:], in_=ids_ext[:, :])
            cb_t = qs_t = None
            if include_c:
                cb_t = const.tile([P, D], F32)
                nc.sync.dma_start(out=cb_t[:], in_=cb_ext[:, :])
                qs_t = const.tile([P, nch], F32)
                nc.sync.dma_start(out=qs_t[:], in_=qs_ext[:, :])

            loop_ctx = (tc.For_i(0, loop_n, 1) if loop_n > 1
                        else contextlib.nullcontext())
            with loop_ctx:
                # --- phase 1: h = LN(x), bf16, tiled layout, to HBM ---
                for t0 in range(0, nt, GT):
                    g_n = min(GT, nt - t0)
                    x8 = xp.tile([P, GT, D], BF16)
                    nc.sync.dma_start(out=x8[:, :g_n, :],
                                      in_=xq_ext[:, t0:t0 + g_n, :])
                    # grouped bn_stats: [P, 4, 128] -> [P, 4, 6]
                    # 6-tuple = (n_e, mean_e, n_e*var_e, n_o, mean_o,
                    # n_o*var_o) over even/odd elements (64 each); combine:
                    # mean = (m_e+m_o)/2, var = (s2_e+s2_o)/128 + (m_e-m_o)^2/4
                    # walrus rejects multi-group BNStats (out must be 6
                    # elems/partition) -> one bn_stats per tile, shared tile
                    st8 = sp.tile([P, GT, 6], F32, tag="st")
                    for j in range(g_n):
                        nc.vector.bn_stats(out=st8[:, j, :],
                                           in_=x8[:, j, :])
                    me = st8[:, :g_n, 1]
                    mo = st8[:, :g_n, 4]
                    s8 = smp.tile([P, GT], F32, tag="s8")
                    nc.gpsimd.tensor_tensor(out=s8[:, :g_n], in0=me, in1=mo,
                                            op=OP.add)
                    d8 = smp.tile([P, GT], F32, tag="d8")
                    nc.gpsimd.tensor_tensor(out=d8[:, :g_n], in0=me, in1=mo,
                                            op=OP.subtract)
                    v8 = smp.tile([P, GT], F32, tag="v8")
                    nc.gpsimd.tensor_tensor(out=v8[:, :g_n],
                                            in0=st8[:, :g_n, 2],
                                            in1=st8[:, :g_n, 5], op=OP.add)
                    u8 = smp.tile([P, GT], F32, tag="u8")
                    nc.vector.scalar_tensor_tensor(
                        out=u8[:, :g_n], in0=d8[:, :g_n], scalar=0.25,
                        in1=d8[:, :g_n], op0=OP.mult, op1=OP.mult)
                    var8 = smp.tile([P, GT], F32, tag="var8")
                    nc.vector.scalar_tensor_tensor(
                        out=var8[:, :g_n], in0=v8[:, :g_n], scalar=1.0 / 128,
                        in1=u8[:, :g_n], op0=OP.mult, op1=OP.add)
                    sd8 = smp.tile([P, GT], F32, tag="sd")
                    nc.scalar.activation(out=sd8[:, :g_n], in_=var8[:, :g_n],
                                         func=AF.Sqrt, bias=eps_t[:, :],
                                         scale=1.0)
                    rstd8 = smp.tile([P, GT], F32, tag="rstd")
                    nc.vector.reciprocal(out=rstd8[:, :g_n],
                                         in_=sd8[:, :g_n])
                    bias8 = smp.tile([P, GT], F32, tag="bias")
                    nc.vector.scalar_tensor_tensor(
                        out=bias8[:, :g_n], in0=s8[:, :g_n], scalar=-0.5,
                        in1=rstd8[:, :g_n], op0=OP.mult, op1=OP.mult)
                    h8 = hp.tile([P, GT, D], BF16)
                    for j in range(g_n):
                        nc.gpsimd.tensor_scalar(out=h8[:, j, :],
                                                in0=x8[:, j, :],
                                                scalar1=rstd8[:, j:j + 1],
                                                scalar2=bias8[:, j:j + 1],
                                                op0=OP.mult, op1=OP.add)
                    if t0 < ntL:
                        h_dst = hlo_dram.rearrange("(p t) d -> p t d", p=P)
                        nc.scalar.dma_start(out=h_dst[:, t0:t0 + g_n, :],
                                            in_=h8[:, :g_n, :])
                    else:
                        h_dst = hhi_dram.rearrange("(p t) d -> p t d", p=P)
                        nc.scalar.dma_start(
                            out=h_dst[:, t0 - ntL:t0 - ntL + g_n, :],
                            in_=h8[:, :g_n, :])

                # --- phase 2: gather + scatter-matmul per chunk ---
                if debug_phase == 1:
                    for ci in range(nch):
                        xr = epp.tile([P, D], BF16, tag="xr")
                        nc.sync.dma_start(
                            out=xr[:],
                            in_=xres_ext[ci * P:(ci + 1) * P, :])
                        nc.sync.dma_start(out=y_ext[ci * P:(ci + 1) * P, :],
                                          in_=xr[:])
                for g, (c_lo, c_hi) in enumerate(groups):
                    if debug_phase == 1:
                        break
                    nbl = gNL[g]
                    nbh = gNH[g]
                    bt_g = int(coff[c_hi] - coff[c_lo])
                    rl = metp.tile([P, bt_g], F32, tag="rel")
                    nc.sync.dma_start(out=rl[:],
                                      in_=rel_ext[:, coff[c_lo]:coff[c_hi]])
                    wg = metp.tile([P, bt_g], F32, tag="wgt")
                    nc.sync.dma_start(out=wg[:],
                                      in_=wgt_ext[:, coff[c_lo]:coff[c_hi]])
                    msgL = msgHq = None
                    if nbl:
                        ilt = metp.tile([P, nbl * 8], I16, tag="il")
                        nc.sync.dma_start(
                            out=ilt[:],
                            in_=il_ext[:, ioffL[g]:ioffL[g + 1]])
                        msgL = msgLp.tile([P, nbl, D], BF16)
                        nc.gpsimd.dma_gather(
                            msgL[:], hlo_dram[:, :], ilt[:],
                            nbl * 128, nbl * 128, D,
                            queue_num=0, single_packet=False)
                    if nbh:
                        iht = metp.tile([P, nbh * 8], I16, tag="ih")
                        nc.sync.dma_start(
                            out=iht[:],
                            in_=ih_ext[:, ioffH[g]:ioffH[g + 1]])
                        msgHq = msgHp.tile([P, nbh, D], BF16)
                        nc.gpsimd.dma_gather(
                            msgHq[:], hhi_dram[:, :], iht[:],
                            nbh * 128, nbh * 128, D,
                            queue_num=0, single_packet=False)

                    blkL = 0
                    blkH = 0
                    if debug_phase == 2:
                        for ci in range(c_lo, c_hi):
                            xr = epp.tile([P, D], BF16, tag="xr")
                            src_t = msgL if msgL is not None else msgHq
                            nc.vector.tensor_copy(out=xr[:],
                                                  in_=src_t[:, 0, :])
                            nc.sync.dma_start(
                                out=y_ext[ci * P:(ci + 1) * P, :], in_=xr[:])
                        continue
                    for ci in range(c_lo, c_hi):
                        nL, nH = bl[ci], bh[ci]
                        nb_c = nL + nH
                        if nb_c == 0:
                            continue
                        col0 = int(coff[ci] - coff[c_lo])
                        ut_ps = ps_u.tile([P, D], F32)
                        for k in range(nb_c):
                            col = col0 + k
                            oh = ohp.tile([P, P], BF16)
                            oh_eng = nc.vector
                            oh_eng.tensor_scalar(out=oh[:], in0=iota_t[:],
                                                scalar1=rl[:, col:col + 1],
                                                scalar2=wg[:, col:col + 1],
                                                op0=OP.is_equal,
                                                op1=OP.mult)
                            msrc = (msgL[:, blkL + k, :] if k < nL
                                    else msgHq[:, blkH + k - nL, :])
                            nc.tensor.matmul(out=ut_ps[:], lhsT=msrc,
                                             rhs=oh[:], start=(k == 0),
                                             stop=(k == nb_c - 1))
                        blkL += nL
                        blkH += nH

                        ut_sb = utp.tile([P, D], BF16)
                        nc.scalar.copy(out=ut_sb[:], in_=ut_ps[:])
                        o2_ps = ps_o.tile([P, D], F32)
                        nc.tensor.matmul(out=o2_ps[:], lhsT=ut_sb[:],
                                         rhs=w2_t[:], start=True, stop=True)
                        sc = epp.tile([P, D], F32, tag="sc")
                        nc.scalar.activation(out=sc[:], in_=o2_ps[:],
                                             func=AF.Copy,
                                             scale=ids_t[:, ci:ci + 1])
                        if include_c:
                            cc = epp.tile([P, D], F32, tag="cc")
                            nc.vector.tensor_scalar(out=cc[:], in0=cb_t[:],
                                                    scalar1=qs_t[:, ci:ci + 1],
                                                    scalar2=None, op0=OP.mult)
                            nc.vector.tensor_tensor(out=sc[:], in0=sc[:],
                                                    in1=cc[:], op=OP.add)
                        gl = epp.tile([P, D], F32, tag="gl")
                        if af_gelu == "tanh1":
                            nc.scalar.activation(out=gl[:], in_=sc[:],
                                                 func=AF.Tanh)
                        elif af_gelu:
                            nc.scalar.activation(out=gl[:], in_=sc[:],
                                                 func=AF.Gelu)
                        else:
                            # tanh-gelu composition (CoreSim lacks Gelu table)
                            sq = epp.tile([P, D], F32, tag="sq")
                            nc.vector.tensor_mul(out=sq[:], in0=sc[:],
                                                 in1=sc[:])
                            cu = epp.tile([P, D], F32, tag="cu")
                            nc.vector.tensor_mul(out=cu[:], in0=sq[:],
                                                 in1=sc[:])
                            u = epp.tile([P, D], F32, tag="u")
                            nc.vector.tensor_scalar(out=u[:], in0=cu[:],
                                                    scalar1=0.044715,
                                                    scalar2=None, op0=OP.mult)
                            nc.vector.tensor_add(out=u[:], in0=u[:], in1=sc[:])
                            v = epp.tile([P, D], F32, tag="v")
                            nc.scalar.activation(out=v[:], in_=u[:],
                                                 func=AF.Tanh,
                                                 scale=0.7978845608028654)
                            w1 = epp.tile([P, D], F32, tag="w1")
                            nc.vector.tensor_mul(out=w1[:], in0=sc[:], in1=v[:])
                            nc.vector.tensor_add(out=w1[:], in0=w1[:],
                                                 in1=sc[:])
                            nc.vector.tensor_scalar(out=gl[:], in0=w1[:],
                                                    scalar1=0.5, scalar2=None,
                                                    op0=OP.mult)
                        xr = epp.tile([P, D], BF16, tag="xr")
                        nc.sync.dma_start(
                            out=xr[:],
                            in_=xres_ext[ci * P:(ci + 1) * P, :])
                        yt = epp.tile([P, D], BF16, tag="yt")
                        nc.gpsimd.tensor_tensor(out=yt[:], in0=gl[:],
                                                in1=xr[:], op=OP.add)
                        nc.sync.dma_start(out=y_ext[ci * P:(ci + 1) * P, :],
                                          in_=yt[:])

    return nc


def prepare_inputs(x, gamma, beta, W, b, edge_index, edge_weight, dst_scale,
                   n_cores, gc=7):
    """Host-side prep: tile/cast x, fold params, sort edges by (core,
    chunk, region, src), build gather indices + one-hot metadata."""
    N = x.shape[0]
    R = n_cores
    npc = (N + R - 1) // R
    nch = (npc + P - 1) // P
    npc_pad = nch * P
    n_pad = (((R - 1) * npc + npc_pad + P - 1) // P) * P
    nt = n_pad // P
    ntL = REG0_TILES
    ntH = nt - ntL
    NLO = ntL * P

    x = np.asarray(x, np.float32)
    x_pad = np.zeros((n_pad, D), np.float32)
    x_pad[:N] = x
    # tiled bf16 layout xq[p, t, :] = x[t*128 + p]
    xq = np.ascontiguousarray(
        x_pad.reshape(nt, P, D).transpose(1, 0, 2)).astype(ml_dtypes.bfloat16)

    W2 = (np.asarray(W, np.float32).T * np.asarray(gamma, np.float32)[:, None])
    W2 = np.ascontiguousarray(W2).astype(ml_dtypes.bfloat16)
    c = (np.asarray(beta, np.float32) @ np.asarray(W, np.float32).T
         + np.asarray(b, np.float32))
    include_c = bool(np.any(c != 0.0))
    cb = np.ascontiguousarray(np.broadcast_to(c, (P, D))).astype(np.float32)

    iota = np.broadcast_to(np.arange(P, dtype=np.float32), (P, P))
    iota = np.ascontiguousarray(iota).astype(ml_dtypes.bfloat16)

    src = np.asarray(edge_index[0]).astype(np.int64)
    dst = np.asarray(edge_index[1]).astype(np.int64)
    w = np.asarray(edge_weight, np.float32)

    deg = np.bincount(dst, weights=w.astype(np.float64),
                      minlength=N).astype(np.float32)
    dmax = np.maximum(deg, 1.0)
    invd = np.asarray(dst_scale, np.float32) / dmax
    qd = deg / dmax  # deg/max(deg,1): multiplier for the c term

    core_id = np.minimum(dst // npc, R - 1)
    local = dst - core_id * npc
    chunk_id = local // P
    rel = (local % P).astype(np.float32)
    region = (src >= NLO).astype(np.int64)
    # tiled flat row index within region
    idx = np.where(region == 0,
                   (src % P) * ntL + src // P,
                   (src % P) * ntH + (src - NLO) // P).astype(np.int64)

    # sort by (core, chunk, region, src) for gather locality
    key = ((core_id * nch + chunk_id) * 2 + region) * (1 << 17) + src
    order = np.argsort(key, kind="stable")
    idx_s = idx[order]
    rel_s = rel[order]
    w_s = w[order]
    seg = ((core_id * nch + chunk_id) * 2 + region)[order]

    nseg = R * nch * 2
    cnt = np.bincount(seg, minlength=nseg).reshape(R, nch, 2)
    blocks = -(-cnt // P)  # ceil
    bl = blocks[:, :, 0].max(axis=0)
    bh = blocks[:, :, 1].max(axis=0)
    # every chunk needs >= 1 block total for PSUM start/stop
    empty = (bl + bh) == 0
    bl[empty] = 1
    bl = tuple(int(v) for v in bl)
    bh = tuple(int(v) for v in bh)
    btot = sum(bl) + sum(bh)
    coff = np.concatenate([[0], np.cumsum(np.array(bl) + np.array(bh))])

    groups = []
    a = 0
    while a < nch:
        groups.append((a, min(a + gc, nch)))
        a += gc
    groups = tuple(groups)
    gNL = [sum(bl[a:b]) for a, b in groups]
    gNH = [sum(bh[a:b]) for a, b in groups]
    ioffL = np.concatenate([[0], np.cumsum([n * 8 for n in gNL])]).astype(int)
    ioffH = np.concatenate([[0], np.cumsum([n * 8 for n in gNH])]).astype(int)

    seg_starts = np.searchsorted(seg, np.arange(nseg + 1))
    pos_in_seg = np.arange(len(seg)) - seg_starts[seg]

    in_maps = []
    for r in range(R):
        rel_arr = np.zeros((P, btot), np.float32)
        w_arr = np.zeros((P, btot), np.float32)
        il_arr = np.zeros((P, max(int(ioffL[-1]), 1)), np.int16)
        ih_arr = np.zeros((P, max(int(ioffH[-1]), 1)), np.int16)

        for g, (c_lo, c_hi) in enumerate(groups):
            # index lists per group: concat chunks, LO then HI separately
            posL = 0
            posH = 0
            for ci in range(c_lo, c_hi):
                for reg, nb in ((0, bl[ci]), (1, bh[ci])):
                    if nb == 0:
                        continue
                    s0 = seg_starts[(r * nch + ci) * 2 + reg]
                    s1 = seg_starts[(r * nch + ci) * 2 + reg + 1]
                    k = s1 - s0
                    ids = np.zeros(nb * P, np.int16)
                    ids[:k] = idx_s[s0:s1]
                    rr = np.zeros(nb * P, np.float32)
                    rr[:k] = rel_s[s0:s1]
                    ww = np.zeros(nb * P, np.float32)
                    ww[:k] = w_s[s0:s1]
                    # rel/w: [P, nb] with edge i -> partition i%128, col i//128
                    cbase = int(coff[ci]) + (0 if reg == 0 else bl[ci])
                    rel_arr[:, cbase:cbase + nb] = rr.reshape(nb, P).T
                    w_arr[:, cbase:cbase + nb] = ww.reshape(nb, P).T
                    # idx: 16-wrap, replicated across 8 partition groups
                    cols = nb * 8
                    iwrap = ids.reshape(cols, 16).T  # [16, cols]
                    if reg == 0:
                        o = int(ioffL[g]) + posL
                        il_arr[:, o:o + cols] = np.tile(iwrap, (8, 1))
                        posL += cols
                    else:
                        o = int(ioffH[g]) + posH
                        ih_arr[:, o:o + cols] = np.tile(iwrap, (8, 1))
                        posH += cols

        lo = r * npc
        hi = min(N, lo + npc)
        ids_r = np.zeros(npc_pad, np.float32)
        ids_r[:hi - lo] = invd[lo:hi]
        ids_rc = np.ascontiguousarray(ids_r.reshape(nch, P).T)
        xres = np.zeros((npc_pad, D), np.float32)
        xres[:hi - lo] = x_pad[lo:hi]
        xres = xres.astype(ml_dtypes.bfloat16)

        m = {
            "xq": xq,
            "xres": xres,
            "w2": W2,
            "iota": iota,
            "rel": rel_arr,
            "wgt": w_arr,
            "idxlo": il_arr,
            "idxhi": ih_arr,
            "invdsct": ids_rc,
        }
        if include_c:
            qr = np.zeros(npc_pad, np.float32)
            qr[:hi - lo] = (qd * np.asarray(dst_scale, np.float32))[lo:hi]
            m["cb"] = cb
            m["qsct"] = np.ascontiguousarray(qr.reshape(nch, P).T)
        in_maps.append(m)

    geom = dict(nt=nt, bl=bl, bh=bh, groups=groups, include_c=include_c,
                npc=npc, npc_pad=npc_pad, N=N, R=R, nch=nch)
    return in_maps, geom


_PROGRAM_CACHE = {}


def kernel(x, gamma, beta, W, b, edge_index, num_nodes, edge_weight,
           dst_scale, n_cores=8, _collect=None):
    x = np.asarray(x)
    N = x.shape[0]
    in_maps, geom = prepare_inputs(
        np.asarray(x), np.asarray(gamma), np.asarray(beta), np.asarray(W),
        np.asarray(b), np.asarray(edge_index), np.asarray(edge_weight),
        np.asarray(dst_scale), n_cores)

    key = (geom["nt"], geom["bl"], geom["bh"], geom["groups"],
           geom["include_c"])
    nc = _PROGRAM_CACHE.get(key)
    if nc is None:
        nc = build_program(*key)
        nc.finalize()
        _PROGRAM_CACHE[key] = nc

    res = run_bass_kernel_spmd(nc, in_maps, list(range(n_cores)),
                               **(_collect.pop("kwargs") if _collect else {}))
    if _collect is not None:
        _collect["res"] = res

    y = np.empty((N, D), np.float32)
    npc = geom["npc"]
    for r in range(geom["R"]):
        lo = r * npc
        hi = min(N, lo + npc)
        y[lo:hi] = res.results[r]["y"][:hi - lo].astype(np.float32)
    return y


# revision 6
# speedup vs baseline: 5.5854x; 5.5854x over previous
"""Trainium2 Bass kernel for nn_DiWeightedGCNLayer (8-core SPMD), v3.

Math (per reference):
    h   = (x - mu) * rsqrt(var + eps)            # LN, gamma folded into W2
    m   = (h * gamma) @ W.T + b = h @ W2 (+ c)
    out = segsum(m[src] * w, dst) / max(deg, 1) * dst_scale
    y   = x + gelu(out)

Key structure (v3):
  - Matmul associativity: sum_e w_e * (h[src_e] @ W2) = (sum_e w_e *
    onehot_e (x) h[src_e]) @ W2.  Phase 1 only computes/stores h (LN) in
    bf16; phase 2 gathers h rows, accumulates U^T = sum_b msg_b^T @ oh_b
    per 128-dst chunk in PSUM, then applies W2 once per chunk.
  - Gathers use batched InstDMAGatherAnt (dma_gather): one instruction per
    (group-of-chunks, src-region) instead of one indirect DMA per 128
    edges (SWDGE fixed cost ~1us each was the old bottleneck).
  - dma_gather indices are int16: h is stored in two region tensors with
    tiled layout hq[p, t, :] = h[t*128 + p], so the flat row index
    (s%128)*ntile + s//128 stays < 32768 for both regions.
  - x input is pre-tiled/bf16 on the host (xq[p, t, :] = x[t*128+p]) so
    every load/store is 2KB-contiguous per partition.
  - deg never computed on device: host folds dst_scale/max(deg,1) into a
    per-node scalar.
Engine balance: DVE bn_stats/bn_aggr + one-hots; Pool normalize +
gathers; ACT rsqrt + h-stores + U copies + gelu; PE scatter matmuls; SP
x/meta/xres/y DMAs.
"""

import contextlib
import numpy as np
import ml_dtypes

import concourse.bass as bass
import concourse.bacc as bacc
import concourse.tile as tile
import concourse.mybir as mybir
from concourse.bass_utils import run_bass_kernel_spmd

F32 = mybir.dt.float32
BF16 = mybir.dt.bfloat16
I16 = mybir.dt.int16
AF = mybir.ActivationFunctionType
OP = mybir.AluOpType

D = 128
P = 128
LN_EPS = 1e-5
REG0_TILES = 256          # LO region = nodes [0, 256*128) = [0, 32768)
GT = 8                    # phase-1 tiles per x-load/h-store group


def build_program(nt, bl, bh, groups, include_c=False, af_gelu=True,
                  loop_n=1, gq=4, debug_phase=0):
    """One-core SPMD program.

    nt: total 128-row node tiles (incl. padding).
    bl/bh: per chunk-index LO/HI block counts (tuples, len nch).
    groups: tuple of (chunk_lo, chunk_hi) chunk-index ranges per gather
        group.
    """
    nch = len(bl)
    ntL = REG0_TILES
    ntH = nt - REG0_TILES
    btot = sum(bl) + sum(bh)
    # per-chunk column offset into rel/w arrays
    coff = np.concatenate([[0], np.cumsum(np.array(bl) + np.array(bh))])
    # per-group idx column offsets (int16 cols = n_idx/16 = blocks*8)
    gNL = [sum(bl[a:b]) for a, b in groups]
    gNH = [sum(bh[a:b]) for a, b in groups]
    ioffL = np.concatenate([[0], np.cumsum([n * 8 for n in gNL])])
    ioffH = np.concatenate([[0], np.cumsum([n * 8 for n in gNH])])

    nc = bacc.Bacc(num_swdge_queues=gq)

    xq_ext = nc.declare_dram_parameter("xq", [P, nt, D], BF16, isOutput=False)
    xres_ext = nc.declare_dram_parameter("xres", [nch * P, D], BF16,
                                         isOutput=False)
    w2_ext = nc.declare_dram_parameter("w2", [D, D], BF16, isOutput=False)
    iota_ext = nc.declare_dram_parameter("iota", [P, P], BF16, isOutput=False)
    rel_ext = nc.declare_dram_parameter("rel", [P, btot], F32, isOutput=False)
    wgt_ext = nc.declare_dram_parameter("wgt", [P, btot], F32, isOutput=False)
    il_ext = nc.declare_dram_parameter("idxlo", [P, max(int(ioffL[-1]), 1)],
                                       I16, isOutput=False)
    ih_ext = nc.declare_dram_parameter("idxhi", [P, max(int(ioffH[-1]), 1)],
                                       I16, isOutput=False)
    ids_ext = nc.declare_dram_parameter("invdsct", [P, nch], F32,
                                        isOutput=False)
    if include_c:
        cb_ext = nc.declare_dram_parameter("cb", [P, D], F32, isOutput=False)
        qs_ext = nc.declare_dram_parameter("qsct", [P, nch], F32,
                                           isOutput=False)
    y_ext = nc.declare_dram_parameter("y", [nch * P, D], BF16, isOutput=True)

    hlo_dram = nc.dram_tensor("h_lo", [P * ntL, D], BF16)
    hhi_dram = nc.dram_tensor("h_hi", [P * ntH, D], BF16)

    with tile.TileContext(nc) as tc:
        with (
            tc.tile_pool(name="const", bufs=1) as const,
            tc.tile_pool(name="xp", bufs=4) as xp,
            tc.tile_pool(name="hp", bufs=4) as hp,
            tc.tile_pool(name="stats", bufs=5) as sp,
            tc.tile_pool(name="small", bufs=8) as smp,
            tc.tile_pool(name="msgL", bufs=3) as msgLp,
            tc.tile_pool(name="msgH", bufs=3) as msgHp,
            tc.tile_pool(name="meta", bufs=3) as metp,
            tc.tile_pool(name="oh", bufs=130) as ohp,
            tc.tile_pool(name="ut", bufs=4) as utp,
            tc.tile_pool(name="ep", bufs=8) as epp,
            tc.tile_pool(name="ps_u", bufs=6, space="PSUM") as ps_u,
            tc.tile_pool(name="ps_o", bufs=2, space="PSUM") as ps_o,
        ):
            # --- constants ---
            w2_t = const.tile([D, D], BF16)
            nc.sync.dma_start(out=w2_t[:], in_=w2_ext[:, :])
            iota_t = const.tile([P, P], BF16)
            nc.sync.dma_start(out=iota_t[:], in_=iota_ext[:, :])
            eps_t = const.tile([P, 1], F32)
            nc.vector.memset(eps_t[:], LN_EPS)
            ids_t = const.tile([P, nch], F32)
            nc.sync.dma_start(out=ids_t[:], in_=ids_ext[:, :])
            cb_t = qs_t = None
            if include_c:
                cb_t = const.tile([P, D], F32)
                nc.sync.dma_start(out=cb_t[:], in_=cb_ext[:, :])
                qs_t = const.tile([P, nch], F32)
                nc.sync.dma_start(out=qs_t[:], in_=qs_ext[:, :])

            loop_ctx = (tc.For_i(0, loop_n, 1) if loop_n > 1
                        else contextlib.nullcontext())
            with loop_ctx:
                # --- phase 1: h = LN(x), bf16, tiled layout, to HBM ---
                for t0 in range(0, nt, GT):
                    g_n = min(GT, nt - t0)
                    x8 = xp.tile([P, GT, D], BF16)
                    nc.sync.dma_start(out=x8[:, :g_n, :],
                                      in_=xq_ext[:, t0:t0 + g_n, :])
                    # grouped bn_stats: [P, 4, 128] -> [P, 4, 6]
                    # 6-tuple = (n_e, mean_e, n_e*var_e, n_o, mean_o,
                    # n_o*var_o) over even/odd elements (64 each); combine:
                    # mean = (m_e+m_o)/2, var = (s2_e+s2_o)/128 + (m_e-m_o)^2/4
                    # walrus rejects multi-group BNStats (out must be 6
                    # elems/partition) -> one bn_stats per tile, shared tile
                    st8 = sp.tile([P, GT, 6], F32, tag="st")
                    for j in range(g_n):
                        nc.vector.bn_stats(out=st8[:, j, :],
                                           in_=x8[:, j, :])
                    me = st8[:, :g_n, 1]
                    mo = st8[:, :g_n, 4]
                    s8 = smp.tile([P, GT], F32, tag="s8")
                    nc.vector.tensor_tensor(out=s8[:, :g_n], in0=me, in1=mo,
                                            op=OP.add)
                    d8 = smp.tile([P, GT], F32, tag="d8")
                    nc.vector.tensor_tensor(out=d8[:, :g_n], in0=me, in1=mo,
                                            op=OP.subtract)
                    v8 = smp.tile([P, GT], F32, tag="v8")
                    nc.vector.tensor_tensor(out=v8[:, :g_n],
                                            in0=st8[:, :g_n, 2],
                                            in1=st8[:, :g_n, 5], op=OP.add)
                    u8 = smp.tile([P, GT], F32, tag="u8")
                    nc.vector.scalar_tensor_tensor(
                        out=u8[:, :g_n], in0=d8[:, :g_n], scalar=0.25,
                        in1=d8[:, :g_n], op0=OP.mult, op1=OP.mult)
                    var8 = smp.tile([P, GT], F32, tag="var8")
                    nc.vector.scalar_tensor_tensor(
                        out=var8[:, :g_n], in0=v8[:, :g_n], scalar=1.0 / 128,
                        in1=u8[:, :g_n], op0=OP.mult, op1=OP.add)
                    sd8 = smp.tile([P, GT], F32, tag="sd")
                    nc.scalar.activation(out=sd8[:, :g_n], in_=var8[:, :g_n],
                                         func=AF.Sqrt, bias=eps_t[:, :],
                                         scale=1.0)
                    rstd8 = smp.tile([P, GT], F32, tag="rstd")
                    nc.vector.reciprocal(out=rstd8[:, :g_n],
                                         in_=sd8[:, :g_n])
                    bias8 = smp.tile([P, GT], F32, tag="bias")
                    nc.vector.scalar_tensor_tensor(
                        out=bias8[:, :g_n], in0=s8[:, :g_n], scalar=-0.5,
                        in1=rstd8[:, :g_n], op0=OP.mult, op1=OP.mult)
                    h8 = hp.tile([P, GT, D], BF16)
                    for j in range(g_n):
                        nc.vector.tensor_scalar(out=h8[:, j, :],
                                                in0=x8[:, j, :],
                                                scalar1=rstd8[:, j:j + 1],
                                                scalar2=bias8[:, j:j + 1],
                                                op0=OP.mult, op1=OP.add)
                    if t0 < ntL:
                        h_dst = hlo_dram.rearrange("(p t) d -> p t d", p=P)
                        nc.scalar.dma_start(out=h_dst[:, t0:t0 + g_n, :],
                                            in_=h8[:, :g_n, :])
                    else:
                        h_dst = hhi_dram.rearrange("(p t) d -> p t d", p=P)
                        nc.scalar.dma_start(
                            out=h_dst[:, t0 - ntL:t0 - ntL + g_n, :],
                            in_=h8[:, :g_n, :])

                # --- phase 2: gather + scatter-matmul per chunk ---
                if debug_phase == 1:
                    for ci in range(nch):
                        xr = epp.tile([P, D], BF16, tag="xr")
                        nc.sync.dma_start(
                            out=xr[:],
                            in_=xres_ext[ci * P:(ci + 1) * P, :])
                        nc.sync.dma_start(out=y_ext[ci * P:(ci + 1) * P, :],
                                          in_=xr[:])
                for g, (c_lo, c_hi) in enumerate(groups):
                    if debug_phase == 1:
                        break
                    nbl = gNL[g]
                    nbh = gNH[g]
                    bt_g = int(coff[c_hi] - coff[c_lo])
                    rl = metp.tile([P, bt_g], F32, tag="rel")
                    nc.sync.dma_start(out=rl[:],
                                      in_=rel_ext[:, coff[c_lo]:coff[c_hi]])
                    wg = metp.tile([P, bt_g], F32, tag="wgt")
                    nc.sync.dma_start(out=wg[:],
                                      in_=wgt_ext[:, coff[c_lo]:coff[c_hi]])
                    msgL = msgHq = None
                    if nbl:
                        ilt = metp.tile([P, nbl * 8], I16, tag="il")
                        nc.sync.dma_start(
                            out=ilt[:],
                            in_=il_ext[:, ioffL[g]:ioffL[g + 1]])
                        msgL = msgLp.tile([P, nbl, D], BF16)
                        hb = max(1, nbl // 2)
                        nc.gpsimd.dma_gather(
                            msgL[:, :hb, :], hlo_dram[:, :], ilt[:, :hb * 8],
                            hb * 128, hb * 128, D,
                            queue_num=0, single_packet=False)
                        if nbl > hb:
                            nc.gpsimd.dma_gather(
                                msgL[:, hb:, :], hlo_dram[:, :],
                                ilt[:, hb * 8:],
                                (nbl - hb) * 128, (nbl - hb) * 128, D,
                                queue_num=0, single_packet=False)
                    if nbh:
                        iht = metp.tile([P, nbh * 8], I16, tag="ih")
                        nc.sync.dma_start(
                            out=iht[:],
                            in_=ih_ext[:, ioffH[g]:ioffH[g + 1]])
                        msgHq = msgHp.tile([P, nbh, D], BF16)
                        hbh = max(1, nbh // 2)
                        nc.gpsimd.dma_gather(
                            msgHq[:, :hbh, :], hhi_dram[:, :], iht[:, :hbh * 8],
                            hbh * 128, hbh * 128, D,
                            queue_num=0, single_packet=False)
                        if nbh > hbh:
                            nc.gpsimd.dma_gather(
                                msgHq[:, hbh:, :], hhi_dram[:, :],
                                iht[:, hbh * 8:],
                                (nbh - hbh) * 128, (nbh - hbh) * 128, D,
                                queue_num=0, single_packet=False)

                    n_groups = len(groups)
                    oh_pool_g = False
                    blkL = 0
                    blkH = 0
                    if debug_phase == 2:
                        for ci in range(c_lo, c_hi):
                            xr = epp.tile([P, D], BF16, tag="xr")
                            src_t = msgL if msgL is not None else msgHq
                            nc.vector.tensor_copy(out=xr[:],
                                                  in_=src_t[:, 0, :])
                            nc.sync.dma_start(
                                out=y_ext[ci * P:(ci + 1) * P, :], in_=xr[:])
                        continue
                    for ci in range(c_lo, c_hi):
                        nL, nH = bl[ci], bh[ci]
                        nb_c = nL + nH
                        if nb_c == 0:
                            continue
                        col0 = int(coff[ci] - coff[c_lo])
                        ut_ps = ps_u.tile([P, D], F32)
                        for k in range(nb_c):
                            col = col0 + k
                            oh = ohp.tile([P, P], BF16)
                            oh_eng = (nc.gpsimd
                                      if (oh_pool_g and col % 2 == 0)
                                      else nc.vector)
                            oh_eng.tensor_scalar(out=oh[:], in0=iota_t[:],
                                                scalar1=rl[:, col:col + 1],
                                                scalar2=wg[:, col:col + 1],
                                                op0=OP.is_equal,
                                                op1=OP.mult)
                            msrc = (msgL[:, blkL + k, :] if k < nL
                                    else msgHq[:, blkH + k - nL, :])
                            nc.tensor.matmul(out=ut_ps[:], lhsT=msrc,
                                             rhs=oh[:], start=(k == 0),
                                             stop=(k == nb_c - 1))
                        blkL += nL
                        blkH += nH

                        ut_sb = utp.tile([P, D], BF16)
                        nc.scalar.copy(out=ut_sb[:], in_=ut_ps[:])
                        o2_ps = ps_o.tile([P, D], F32)
                        nc.tensor.matmul(out=o2_ps[:], lhsT=ut_sb[:],
                                         rhs=w2_t[:], start=True, stop=True)
                        sc = epp.tile([P, D], F32, tag="sc")
                        nc.scalar.activation(out=sc[:], in_=o2_ps[:],
                                             func=AF.Copy,
                                             scale=ids_t[:, ci:ci + 1])
                        if include_c:
                            cc = epp.tile([P, D], F32, tag="cc")
                            nc.vector.tensor_scalar(out=cc[:], in0=cb_t[:],
                                                    scalar1=qs_t[:, ci:ci + 1],
                                                    scalar2=None, op0=OP.mult)
                            nc.vector.tensor_tensor(out=sc[:], in0=sc[:],
                                                    in1=cc[:], op=OP.add)
                        gl = epp.tile([P, D], F32, tag="gl")
                        if af_gelu == "tanh1":
                            nc.scalar.activation(out=gl[:], in_=sc[:],
                                                 func=AF.Tanh)
                        elif af_gelu:
                            nc.scalar.activation(out=gl[:], in_=sc[:],
                                                 func=AF.Gelu)
                        else:
                            # tanh-gelu composition (CoreSim lacks Gelu table)
                            sq = epp.tile([P, D], F32, tag="sq")
                            nc.vector.tensor_mul(out=sq[:], in0=sc[:],
                                                 in1=sc[:])
                            cu = epp.tile([P, D], F32, tag="cu")
                            nc.vector.tensor_mul(out=cu[:], in0=sq[:],
                                                 in1=sc[:])
                            u = epp.tile([P, D], F32, tag="u")
                            nc.vector.tensor_scalar(out=u[:], in0=cu[:],
                                                    scalar1=0.044715,
                                                    scalar2=None, op0=OP.mult)
                            nc.vector.tensor_add(out=u[:], in0=u[:], in1=sc[:])
                            v = epp.tile([P, D], F32, tag="v")
                            nc.scalar.activation(out=v[:], in_=u[:],
                                                 func=AF.Tanh,
                                                 scale=0.7978845608028654)
                            w1 = epp.tile([P, D], F32, tag="w1")
                            nc.vector.tensor_mul(out=w1[:], in0=sc[:], in1=v[:])
                            nc.vector.tensor_add(out=w1[:], in0=w1[:],
                                                 in1=sc[:])
                            nc.vector.tensor_scalar(out=gl[:], in0=w1[:],
                                                    scalar1=0.5, scalar2=None,
                                                    op0=OP.mult)
                        xr = epp.tile([P, D], BF16, tag="xr")
                        nc.sync.dma_start(
                            out=xr[:],
                            in_=xres_ext[ci * P:(ci + 1) * P, :])
                        yt = epp.tile([P, D], BF16, tag="yt")
                        nc.vector.tensor_tensor(out=yt[:], in0=gl[:],
                                                in1=xr[:], op=OP.add)
                        nc.sync.dma_start(out=y_ext[ci * P:(ci + 1) * P, :],
                                          in_=yt[:])

    return nc


def prepare_inputs(x, gamma, beta, W, b, edge_index, edge_weight, dst_scale,
                   n_cores, gc=7):
    """Host-side prep: tile/cast x, fold params, sort edges by (core,
    chunk, region, src), build gather indices + one-hot metadata."""
    N = x.shape[0]
    R = n_cores
    npc = (N + R - 1) // R
    nch = (npc + P - 1) // P
    npc_pad = nch * P
    n_pad = (((R - 1) * npc + npc_pad + P - 1) // P) * P
    nt = n_pad // P
    ntL = REG0_TILES
    ntH = nt - ntL
    NLO = ntL * P

    x = np.asarray(x, np.float32)
    x_pad = np.zeros((n_pad, D), np.float32)
    x_pad[:N] = x
    # tiled bf16 layout xq[p, t, :] = x[t*128 + p]
    xq = np.ascontiguousarray(
        x_pad.reshape(nt, P, D).transpose(1, 0, 2)).astype(ml_dtypes.bfloat16)

    W2 = (np.asarray(W, np.float32).T * np.asarray(gamma, np.float32)[:, None])
    W2 = np.ascontiguousarray(W2).astype(ml_dtypes.bfloat16)
    c = (np.asarray(beta, np.float32) @ np.asarray(W, np.float32).T
         + np.asarray(b, np.float32))
    include_c = bool(np.any(c != 0.0))
    cb = np.ascontiguousarray(np.broadcast_to(c, (P, D))).astype(np.float32)

    iota = np.broadcast_to(np.arange(P, dtype=np.float32), (P, P))
    iota = np.ascontiguousarray(iota).astype(ml_dtypes.bfloat16)

    src = np.asarray(edge_index[0]).astype(np.int64)
    dst = np.asarray(edge_index[1]).astype(np.int64)
    w = np.asarray(edge_weight, np.float32)

    deg = np.bincount(dst, weights=w.astype(np.float64),
                      minlength=N).astype(np.float32)
    dmax = np.maximum(deg, 1.0)
    invd = np.asarray(dst_scale, np.float32) / dmax
    qd = deg / dmax  # deg/max(deg,1): multiplier for the c term

    core_id = np.minimum(dst // npc, R - 1)
    local = dst - core_id * npc
    chunk_id = local // P
    rel = (local % P).astype(np.float32)
    region = (src >= NLO).astype(np.int64)
    # tiled flat row index within region
    idx = np.where(region == 0,
                   (src % P) * ntL + src // P,
                   (src % P) * ntH + (src - NLO) // P).astype(np.int64)

    # sort by (core, chunk, region, src) for gather locality
    key = ((core_id * nch + chunk_id) * 2 + region) * (1 << 17) + src
    order = np.argsort(key, kind="stable")
    idx_s = idx[order]
    rel_s = rel[order]
    w_s = w[order]
    seg = ((core_id * nch + chunk_id) * 2 + region)[order]

    nseg = R * nch * 2
    cnt = np.bincount(seg, minlength=nseg).reshape(R, nch, 2)
    blocks = -(-cnt // P)  # ceil
    bl = blocks[:, :, 0].max(axis=0)
    bh = blocks[:, :, 1].max(axis=0)
    # every chunk needs >= 1 block total for PSUM start/stop
    empty = (bl + bh) == 0
    bl[empty] = 1
    bl = tuple(int(v) for v in bl)
    bh = tuple(int(v) for v in bh)
    btot = sum(bl) + sum(bh)
    coff = np.concatenate([[0], np.cumsum(np.array(bl) + np.array(bh))])

    groups = []
    a = 0
    while a < nch:
        groups.append((a, min(a + gc, nch)))
        a += gc
    groups = tuple(groups)
    gNL = [sum(bl[a:b]) for a, b in groups]
    gNH = [sum(bh[a:b]) for a, b in groups]
    ioffL = np.concatenate([[0], np.cumsum([n * 8 for n in gNL])]).astype(int)
    ioffH = np.concatenate([[0], np.cumsum([n * 8 for n in gNH])]).astype(int)

    seg_starts = np.searchsorted(seg, np.arange(nseg + 1))
    pos_in_seg = np.arange(len(seg)) - seg_starts[seg]

    in_maps = []
    for r in range(R):
        rel_arr = np.zeros((P, btot), np.float32)
        w_arr = np.zeros((P, btot), np.float32)
        il_arr = np.zeros((P, max(int(ioffL[-1]), 1)), np.int16)
        ih_arr = np.zeros((P, max(int(ioffH[-1]), 1)), np.int16)

        for g, (c_lo, c_hi) in enumerate(groups):
            # index lists per group: concat chunks, LO then HI separately
            posL = 0
            posH = 0
            for ci in range(c_lo, c_hi):
                for reg, nb in ((0, bl[ci]), (1, bh[ci])):
                    if nb == 0:
                        continue
                    s0 = seg_starts[(r * nch + ci) * 2 + reg]
                    s1 = seg_starts[(r * nch + ci) * 2 + reg + 1]
                    k = s1 - s0
                    ids = np.zeros(nb * P, np.int16)
                    ids[:k] = idx_s[s0:s1]
                    rr = np.zeros(nb * P, np.float32)
                    rr[:k] = rel_s[s0:s1]
                    ww = np.zeros(nb * P, np.float32)
                    ww[:k] = w_s[s0:s1]
                    # rel/w: [P, nb] with edge i -> partition i%128, col i//128
                    cbase = int(coff[ci]) + (0 if reg == 0 else bl[ci])
                    rel_arr[:, cbase:cbase + nb] = rr.reshape(nb, P).T
                    w_arr[:, cbase:cbase + nb] = ww.reshape(nb, P).T
                    # idx: 16-wrap, replicated across 8 partition groups
                    cols = nb * 8
                    iwrap = ids.reshape(cols, 16).T  # [16, cols]
                    if reg == 0:
                        o = int(ioffL[g]) + posL
                        il_arr[:, o:o + cols] = np.tile(iwrap, (8, 1))
                        posL += cols
                    else:
                        o = int(ioffH[g]) + posH
                        ih_arr[:, o:o + cols] = np.tile(iwrap, (8, 1))
                        posH += cols

        lo = r * npc
        hi = min(N, lo + npc)
        ids_r = np.zeros(npc_pad, np.float32)
        ids_r[:hi - lo] = invd[lo:hi]
        ids_rc = np.ascontiguousarray(ids_r.reshape(nch, P).T)
        xres = np.zeros((npc_pad, D), np.float32)
        xres[:hi - lo] = x_pad[lo:hi]
        xres = xres.astype(ml_dtypes.bfloat16)

        m = {
            "xq": xq,
            "xres": xres,
            "w2": W2,
            "iota": iota,
            "rel": rel_arr,
            "wgt": w_arr,
            "idxlo": il_arr,
            "idxhi": ih_arr,
            "invdsct": ids_rc,
        }
        if include_c:
            qr = np.zeros(npc_pad, np.float32)
            qr[:hi - lo] = (qd * np.asarray(dst_scale, np.float32))[lo:hi]
            m["cb"] = cb
            m["qsct"] = np.ascontiguousarray(qr.reshape(nch, P).T)
        in_maps.append(m)

    geom = dict(nt=nt, bl=bl, bh=bh, groups=groups, include_c=include_c,
                npc=npc, npc_pad=npc_pad, N=N, R=R, nch=nch)
    return in_maps, geom


def fix_gather_queues(nc, gq=4):
    """Post-finalize: spread InstDMAGatherAnt across SWDGE queues, matching
    the tile scheduler's DMASW lane assignment (queue = lane % gq) so each
    DMA semaphore stays locked to one queue."""
    from concourse.tile_sem_assignment import PROC_NAME_TO_IDX
    lane_of = {PROC_NAME_TO_IDX[f"DMASW{i}"]: i for i in range(8)}
    for block in nc.m.functions[0].blocks:
        for inst in block.instructions:
            if type(inst).__name__ == "InstDMAGatherAnt":
                proc = getattr(inst, "bass_scheduled_proc", None)
                if proc in lane_of:
                    inst.queue_num = lane_of[proc] % gq


_PROGRAM_CACHE = {}


def kernel(x, gamma, beta, W, b, edge_index, num_nodes, edge_weight,
           dst_scale, n_cores=8, _collect=None):
    x = np.asarray(x)
    N = x.shape[0]
    in_maps, geom = prepare_inputs(
        np.asarray(x), np.asarray(gamma), np.asarray(beta), np.asarray(W),
        np.asarray(b), np.asarray(edge_index), np.asarray(edge_weight),
        np.asarray(dst_scale), n_cores)

    key = (geom["nt"], geom["bl"], geom["bh"], geom["groups"],
           geom["include_c"])
    nc = _PROGRAM_CACHE.get(key)
    if nc is None:
        nc = build_program(*key)
        nc.finalize()
        fix_gather_queues(nc)
        _PROGRAM_CACHE[key] = nc

    res = run_bass_kernel_spmd(nc, in_maps, list(range(n_cores)),
                               **(_collect.pop("kwargs") if _collect else {}))
    if _collect is not None:
        _collect["res"] = res

    y = np.empty((N, D), np.float32)
    npc = geom["npc"]
    for r in range(geom["R"]):
        lo = r * npc
        hi = min(N, lo + npc)
        y[lo:hi] = res.results[r]["y"][:hi - lo].astype(np.float32)
    return y


# revision 10
# speedup vs baseline: 7.1701x; 1.2837x over previous
"""Trainium2 Bass kernel for nn_DiWeightedGCNLayer (8-core SPMD), v3.

Math (per reference):
    h   = (x - mu) * rsqrt(var + eps)            # LN, gamma folded into W2
    m   = (h * gamma) @ W.T + b = h @ W2 (+ c)
    out = segsum(m[src] * w, dst) / max(deg, 1) * dst_scale
    y   = x + gelu(out)

Key structure (v3):
  - Matmul associativity: sum_e w_e * (h[src_e] @ W2) = (sum_e w_e *
    onehot_e (x) h[src_e]) @ W2.  Phase 1 only computes/stores h (LN) in
    bf16; phase 2 gathers h rows, accumulates U^T = sum_b msg_b^T @ oh_b
    per 128-dst chunk in PSUM, then applies W2 once per chunk.
  - Gathers use batched InstDMAGatherAnt (dma_gather): one instruction per
    (group-of-chunks, src-region) instead of one indirect DMA per 128
    edges (SWDGE fixed cost ~1us each was the old bottleneck).
  - dma_gather indices are int16: h is stored in two region tensors with
    tiled layout hq[p, t, :] = h[t*128 + p], so the flat row index
    (s%128)*ntile + s//128 stays < 32768 for both regions.
  - x input is pre-tiled/bf16 on the host (xq[p, t, :] = x[t*128+p]) so
    every load/store is 2KB-contiguous per partition.
  - deg never computed on device: host folds dst_scale/max(deg,1) into a
    per-node scalar.
Engine balance: DVE bn_stats/bn_aggr + one-hots; Pool normalize +
gathers; ACT rsqrt + h-stores + U copies + gelu; PE scatter matmuls; SP
x/meta/xres/y DMAs.
"""

import contextlib
import numpy as np
import ml_dtypes

import concourse.bass as bass
import concourse.bacc as bacc
import concourse.tile as tile
import concourse.mybir as mybir
from concourse.bass_utils import run_bass_kernel_spmd

F32 = mybir.dt.float32
BF16 = mybir.dt.bfloat16
I16 = mybir.dt.int16
AF = mybir.ActivationFunctionType
OP = mybir.AluOpType

D = 128
P = 128
LN_EPS = 1e-5
REG0_TILES = 256          # LO region = nodes [0, 256*128) = [0, 32768)
GT = 16                   # phase-1 tiles per x-load/h-store group


def build_program(nt, bl, bh, groups, include_c=False, af_gelu=True,
                  loop_n=1, gq=4, debug_phase=0):
    """One-core SPMD program.

    nt: total 128-row node tiles (incl. padding).
    bl/bh: per chunk-index LO/HI block counts (tuples, len nch).
    groups: tuple of (chunk_lo, chunk_hi) chunk-index ranges per gather
        group.
    """
    nch = len(bl)
    ntL = REG0_TILES
    ntH = nt - REG0_TILES
    btot = sum(bl) + sum(bh)
    # per-chunk column offset into rel/w arrays
    coff = np.concatenate([[0], np.cumsum(np.array(bl) + np.array(bh))])
    # per-group idx column offsets (int16 cols = n_idx/16 = blocks*8)
    gNL = [sum(bl[a:b]) for a, b in groups]
    gNH = [sum(bh[a:b]) for a, b in groups]
    ioffL = np.concatenate([[0], np.cumsum([n * 8 for n in gNL])])
    ioffH = np.concatenate([[0], np.cumsum([n * 8 for n in gNH])])

    nc = bacc.Bacc(num_swdge_queues=gq)

    xq_ext = nc.declare_dram_parameter("xq", [P, nt, D], BF16, isOutput=False)
    xres_ext = nc.declare_dram_parameter("xres", [nch * P, D], BF16,
                                         isOutput=False)
    w2_ext = nc.declare_dram_parameter("w2", [D, D], BF16, isOutput=False)
    iota_ext = nc.declare_dram_parameter("iota", [P, P], BF16, isOutput=False)
    rel_ext = nc.declare_dram_parameter("rel", [P, btot], F32, isOutput=False)
    wgt_ext = nc.declare_dram_parameter("wgt", [P, btot], F32, isOutput=False)
    il_ext = nc.declare_dram_parameter("idxlo", [P, max(int(ioffL[-1]), 1)],
                                       I16, isOutput=False)
    ih_ext = nc.declare_dram_parameter("idxhi", [P, max(int(ioffH[-1]), 1)],
                                       I16, isOutput=False)
    ids_ext = nc.declare_dram_parameter("invdsct", [P, nch], F32,
                                        isOutput=False)
    if include_c:
        cb_ext = nc.declare_dram_parameter("cb", [P, D], F32, isOutput=False)
        qs_ext = nc.declare_dram_parameter("qsct", [P, nch], F32,
                                           isOutput=False)
    y_ext = nc.declare_dram_parameter("y", [nch * P, D], BF16, isOutput=True)

    hlo_dram = nc.dram_tensor("h_lo", [P * ntL, D], BF16)
    hhi_dram = nc.dram_tensor("h_hi", [P * ntH, D], BF16)

    with tile.TileContext(nc) as tc:
        with (
            tc.tile_pool(name="const", bufs=1) as const,
            tc.tile_pool(name="xp", bufs=3) as xp,
            tc.tile_pool(name="hp", bufs=3) as hp,
            tc.tile_pool(name="stats", bufs=8) as sp,
            tc.tile_pool(name="small", bufs=12) as smp,
            tc.tile_pool(name="msgL", bufs=2) as msgLp,
            tc.tile_pool(name="msgH", bufs=2) as msgHp,
            tc.tile_pool(name="meta", bufs=3) as metp,
            tc.tile_pool(name="oh", bufs=100) as ohp,
            tc.tile_pool(name="ut", bufs=4) as utp,
            tc.tile_pool(name="ep", bufs=8) as epp,
            tc.tile_pool(name="ps_u", bufs=6, space="PSUM") as ps_u,
            tc.tile_pool(name="ps_o", bufs=2, space="PSUM") as ps_o,
        ):
            # --- constants ---
            w2_t = const.tile([D, D], BF16)
            nc.sync.dma_start(out=w2_t[:], in_=w2_ext[:, :])
            iota_t = const.tile([P, P], BF16)
            nc.sync.dma_start(out=iota_t[:], in_=iota_ext[:, :])
            eps_t = const.tile([P, 1], F32)
            nc.vector.memset(eps_t[:], LN_EPS)
            ids_t = const.tile([P, nch], F32)
            nc.sync.dma_start(out=ids_t[:], in_=ids_ext[:, :])
            cb_t = qs_t = None
            if include_c:
                cb_t = const.tile([P, D], F32)
                nc.sync.dma_start(out=cb_t[:], in_=cb_ext[:, :])
                qs_t = const.tile([P, nch], F32)
                nc.sync.dma_start(out=qs_t[:], in_=qs_ext[:, :])

            loop_ctx = (tc.For_i(0, loop_n, 1) if loop_n > 1
                        else contextlib.nullcontext())
            with loop_ctx:
                # --- phase 1: h = LN(x), bf16, tiled layout, to HBM ---
                for t0 in range(0, nt, GT):
                    g_n = min(GT, nt - t0)
                    x8 = xp.tile([P, GT, D], BF16)
                    nc.sync.dma_start(out=x8[:, :g_n, :],
                                      in_=xq_ext[:, t0:t0 + g_n, :])
                    # grouped bn_stats: [P, 4, 128] -> [P, 4, 6]
                    # 6-tuple = (n_e, mean_e, n_e*var_e, n_o, mean_o,
                    # n_o*var_o) over even/odd elements (64 each); combine:
                    # mean = (m_e+m_o)/2, var = (s2_e+s2_o)/128 + (m_e-m_o)^2/4
                    # walrus rejects multi-group BNStats (out must be 6
                    # elems/partition) -> one bn_stats per tile, shared tile
                    st8 = sp.tile([P, GT, 6], F32, tag="st")
                    for j in range(g_n):
                        nc.vector.bn_stats(out=st8[:, j, :],
                                           in_=x8[:, j, :])
                    me = st8[:, :g_n, 1]
                    mo = st8[:, :g_n, 4]
                    s8 = smp.tile([P, GT], F32, tag="s8")
                    nc.vector.tensor_tensor(out=s8[:, :g_n], in0=me, in1=mo,
                                            op=OP.add)
                    d8 = smp.tile([P, GT], F32, tag="d8")
                    nc.vector.tensor_tensor(out=d8[:, :g_n], in0=me, in1=mo,
                                            op=OP.subtract)
                    v8 = smp.tile([P, GT], F32, tag="v8")
                    nc.vector.tensor_tensor(out=v8[:, :g_n],
                                            in0=st8[:, :g_n, 2],
                                            in1=st8[:, :g_n, 5], op=OP.add)
                    u8 = smp.tile([P, GT], F32, tag="u8")
                    nc.vector.scalar_tensor_tensor(
                        out=u8[:, :g_n], in0=d8[:, :g_n], scalar=0.25,
                        in1=d8[:, :g_n], op0=OP.mult, op1=OP.mult)
                    var8 = smp.tile([P, GT], F32, tag="var8")
                    nc.vector.scalar_tensor_tensor(
                        out=var8[:, :g_n], in0=v8[:, :g_n], scalar=1.0 / 128,
                        in1=u8[:, :g_n], op0=OP.mult, op1=OP.add)
                    sd8 = smp.tile([P, GT], F32, tag="sd")
                    nc.scalar.activation(out=sd8[:, :g_n], in_=var8[:, :g_n],
                                         func=AF.Sqrt, bias=eps_t[:, :],
                                         scale=1.0)
                    rstd8 = smp.tile([P, GT], F32, tag="rstd")
                    nc.vector.reciprocal(out=rstd8[:, :g_n],
                                         in_=sd8[:, :g_n])
                    bias8 = smp.tile([P, GT], F32, tag="bias")
                    nc.vector.scalar_tensor_tensor(
                        out=bias8[:, :g_n], in0=s8[:, :g_n], scalar=-0.5,
                        in1=rstd8[:, :g_n], op0=OP.mult, op1=OP.mult)
                    h8 = hp.tile([P, GT, D], BF16)
                    for j in range(g_n):
                        nc.vector.tensor_scalar(out=h8[:, j, :],
                                                in0=x8[:, j, :],
                                                scalar1=rstd8[:, j:j + 1],
                                                scalar2=bias8[:, j:j + 1],
                                                op0=OP.mult, op1=OP.add)
                    if t0 < ntL:
                        h_dst = hlo_dram.rearrange("(p t) d -> p t d", p=P)
                        nc.scalar.dma_start(out=h_dst[:, t0:t0 + g_n, :],
                                            in_=h8[:, :g_n, :])
                    else:
                        h_dst = hhi_dram.rearrange("(p t) d -> p t d", p=P)
                        nc.scalar.dma_start(
                            out=h_dst[:, t0 - ntL:t0 - ntL + g_n, :],
                            in_=h8[:, :g_n, :])

                # --- phase 2: gather + scatter-matmul per chunk ---
                if debug_phase == 1:
                    for ci in range(nch):
                        xr = epp.tile([P, D], BF16, tag="xr")
                        nc.sync.dma_start(
                            out=xr[:],
                            in_=xres_ext[ci * P:(ci + 1) * P, :])
                        nc.sync.dma_start(out=y_ext[ci * P:(ci + 1) * P, :],
                                          in_=xr[:])
                for g, (c_lo, c_hi) in enumerate(groups):
                    if debug_phase == 1:
                        break
                    nbl = gNL[g]
                    nbh = gNH[g]
                    bt_g = int(coff[c_hi] - coff[c_lo])
                    rl = metp.tile([P, bt_g], F32, tag="rel")
                    nc.sync.dma_start(out=rl[:],
                                      in_=rel_ext[:, coff[c_lo]:coff[c_hi]])
                    wg = metp.tile([P, bt_g], F32, tag="wgt")
                    nc.sync.dma_start(out=wg[:],
                                      in_=wgt_ext[:, coff[c_lo]:coff[c_hi]])
                    msgL = msgHq = None
                    if nbl:
                        ilt = metp.tile([P, nbl * 8], I16, tag="il")
                        nc.sync.dma_start(
                            out=ilt[:],
                            in_=il_ext[:, ioffL[g]:ioffL[g + 1]])
                        msgL = msgLp.tile([P, nbl, D], BF16)
                        qn = max(1, nbl // 4)
                        b0 = 0
                        while b0 < nbl:
                            b1 = min(b0 + qn, nbl)
                            nc.gpsimd.dma_gather(
                                msgL[:, b0:b1, :], hlo_dram[:, :],
                                ilt[:, b0 * 8:b1 * 8],
                                (b1 - b0) * 128, (b1 - b0) * 128, D,
                                queue_num=0, single_packet=False)
                            b0 = b1
                    if nbh:
                        iht = metp.tile([P, nbh * 8], I16, tag="ih")
                        nc.sync.dma_start(
                            out=iht[:],
                            in_=ih_ext[:, ioffH[g]:ioffH[g + 1]])
                        msgHq = msgHp.tile([P, nbh, D], BF16)
                        qh = max(1, nbh // 2)
                        b0 = 0
                        while b0 < nbh:
                            b1 = min(b0 + qh, nbh)
                            nc.gpsimd.dma_gather(
                                msgHq[:, b0:b1, :], hhi_dram[:, :],
                                iht[:, b0 * 8:b1 * 8],
                                (b1 - b0) * 128, (b1 - b0) * 128, D,
                                queue_num=0, single_packet=False)
                            b0 = b1

                    n_groups = len(groups)
                    oh_pool_g = False
                    blkL = 0
                    blkH = 0
                    if debug_phase == 2:
                        for ci in range(c_lo, c_hi):
                            xr = epp.tile([P, D], BF16, tag="xr")
                            src_t = msgL if msgL is not None else msgHq
                            nc.vector.tensor_copy(out=xr[:],
                                                  in_=src_t[:, 0, :])
                            nc.sync.dma_start(
                                out=y_ext[ci * P:(ci + 1) * P, :], in_=xr[:])
                        continue
                    for ci in range(c_lo, c_hi):
                        nL, nH = bl[ci], bh[ci]
                        nb_c = nL + nH
                        if nb_c == 0:
                            continue
                        col0 = int(coff[ci] - coff[c_lo])
                        ut_ps = ps_u.tile([P, D], F32)
                        for k in range(nb_c):
                            col = col0 + k
                            oh = ohp.tile([P, P], BF16)
                            oh_eng = (nc.gpsimd
                                      if (oh_pool_g and col % 2 == 0)
                                      else nc.vector)
                            oh_eng.tensor_scalar(out=oh[:], in0=iota_t[:],
                                                scalar1=rl[:, col:col + 1],
                                                scalar2=wg[:, col:col + 1],
                                                op0=OP.is_equal,
                                                op1=OP.mult)
                            msrc = (msgL[:, blkL + k, :] if k < nL
                                    else msgHq[:, blkH + k - nL, :])
                            nc.tensor.matmul(out=ut_ps[:], lhsT=msrc,
                                             rhs=oh[:], start=(k == 0),
                                             stop=(k == nb_c - 1))
                        blkL += nL
                        blkH += nH

                        ut_sb = utp.tile([P, D], BF16)
                        nc.scalar.copy(out=ut_sb[:], in_=ut_ps[:])
                        o2_ps = ps_o.tile([P, D], F32)
                        nc.tensor.matmul(out=o2_ps[:], lhsT=ut_sb[:],
                                         rhs=w2_t[:], start=True, stop=True)
                        sc = epp.tile([P, D], F32, tag="sc")
                        nc.vector.tensor_scalar(out=sc[:], in0=o2_ps[:],
                                                scalar1=ids_t[:, ci:ci + 1],
                                                scalar2=None, op0=OP.mult)
                        if include_c:
                            cc = epp.tile([P, D], F32, tag="cc")
                            nc.vector.tensor_scalar(out=cc[:], in0=cb_t[:],
                                                    scalar1=qs_t[:, ci:ci + 1],
                                                    scalar2=None, op0=OP.mult)
                            nc.vector.tensor_tensor(out=sc[:], in0=sc[:],
                                                    in1=cc[:], op=OP.add)
                        gl = epp.tile([P, D], F32, tag="gl")
                        if af_gelu == "tanh1":
                            nc.scalar.activation(out=gl[:], in_=sc[:],
                                                 func=AF.Tanh)
                        elif af_gelu:
                            nc.scalar.activation(out=gl[:], in_=sc[:],
                                                 func=AF.Gelu)
                        else:
                            # tanh-gelu composition (CoreSim lacks Gelu table)
                            sq = epp.tile([P, D], F32, tag="sq")
                            nc.vector.tensor_mul(out=sq[:], in0=sc[:],
                                                 in1=sc[:])
                            cu = epp.tile([P, D], F32, tag="cu")
                            nc.vector.tensor_mul(out=cu[:], in0=sq[:],
                                                 in1=sc[:])
                            u = epp.tile([P, D], F32, tag="u")
                            nc.vector.tensor_scalar(out=u[:], in0=cu[:],
                                                    scalar1=0.044715,
                                                    scalar2=None, op0=OP.mult)
                            nc.vector.tensor_add(out=u[:], in0=u[:], in1=sc[:])
                            v = epp.tile([P, D], F32, tag="v")
                            nc.scalar.activation(out=v[:], in_=u[:],
                                                 func=AF.Tanh,
                                                 scale=0.7978845608028654)
                            w1 = epp.tile([P, D], F32, tag="w1")
                            nc.vector.tensor_mul(out=w1[:], in0=sc[:], in1=v[:])
                            nc.vector.tensor_add(out=w1[:], in0=w1[:],
                                                 in1=sc[:])
                            nc.vector.tensor_scalar(out=gl[:], in0=w1[:],
                                                    scalar1=0.5, scalar2=None,
                                                    op0=OP.mult)
                        xr = epp.tile([P, D], BF16, tag="xr")
                        nc.sync.dma_start(
                            out=xr[:],
                            in_=xres_ext[ci * P:(ci + 1) * P, :])
                        yt = epp.tile([P, D], BF16, tag="yt")
                        nc.vector.tensor_tensor(out=yt[:], in0=gl[:],
                                                in1=xr[:], op=OP.add)
                        nc.sync.dma_start(out=y_ext[ci * P:(ci + 1) * P, :],
                                          in_=yt[:])

    return nc


def prepare_inputs(x, gamma, beta, W, b, edge_index, edge_weight, dst_scale,
                   n_cores, gc=7):
    """Host-side prep: tile/cast x, fold params, sort edges by (core,
    chunk, region, src), build gather indices + one-hot metadata."""
    N = x.shape[0]
    R = n_cores
    npc = (N + R - 1) // R
    nch = (npc + P - 1) // P
    npc_pad = nch * P
    n_pad = (((R - 1) * npc + npc_pad + P - 1) // P) * P
    nt = n_pad // P
    ntL = REG0_TILES
    ntH = nt - ntL
    NLO = ntL * P

    x = np.asarray(x, np.float32)
    x_pad = np.zeros((n_pad, D), np.float32)
    x_pad[:N] = x
    # tiled bf16 layout xq[p, t, :] = x[t*128 + p]
    xq = np.ascontiguousarray(
        x_pad.reshape(nt, P, D).transpose(1, 0, 2)).astype(ml_dtypes.bfloat16)

    W2 = (np.asarray(W, np.float32).T * np.asarray(gamma, np.float32)[:, None])
    W2 = np.ascontiguousarray(W2).astype(ml_dtypes.bfloat16)
    c = (np.asarray(beta, np.float32) @ np.asarray(W, np.float32).T
         + np.asarray(b, np.float32))
    include_c = bool(np.any(c != 0.0))
    cb = np.ascontiguousarray(np.broadcast_to(c, (P, D))).astype(np.float32)

    iota = np.broadcast_to(np.arange(P, dtype=np.float32), (P, P))
    iota = np.ascontiguousarray(iota).astype(ml_dtypes.bfloat16)

    src = np.asarray(edge_index[0]).astype(np.int64)
    dst = np.asarray(edge_index[1]).astype(np.int64)
    w = np.asarray(edge_weight, np.float32)

    deg = np.bincount(dst, weights=w.astype(np.float64),
                      minlength=N).astype(np.float32)
    dmax = np.maximum(deg, 1.0)
    invd = np.asarray(dst_scale, np.float32) / dmax
    qd = deg / dmax  # deg/max(deg,1): multiplier for the c term

    core_id = np.minimum(dst // npc, R - 1)
    local = dst - core_id * npc
    chunk_id = local // P
    rel = (local % P).astype(np.float32)
    region = (src >= NLO).astype(np.int64)
    # tiled flat row index within region
    idx = np.where(region == 0,
                   (src % P) * ntL + src // P,
                   (src % P) * ntH + (src - NLO) // P).astype(np.int64)

    # sort by (core, chunk, region, src) for gather locality
    key = ((core_id * nch + chunk_id) * 2 + region) * (1 << 17) + src
    order = np.argsort(key, kind="stable")
    idx_s = idx[order]
    rel_s = rel[order]
    w_s = w[order]
    seg = ((core_id * nch + chunk_id) * 2 + region)[order]

    nseg = R * nch * 2
    cnt = np.bincount(seg, minlength=nseg).reshape(R, nch, 2)
    blocks = -(-cnt // P)  # ceil
    bl = blocks[:, :, 0].max(axis=0)
    bh = blocks[:, :, 1].max(axis=0)
    # every chunk needs >= 1 block total for PSUM start/stop
    empty = (bl + bh) == 0
    bl[empty] = 1
    bl = tuple(int(v) for v in bl)
    bh = tuple(int(v) for v in bh)
    btot = sum(bl) + sum(bh)
    coff = np.concatenate([[0], np.cumsum(np.array(bl) + np.array(bh))])

    groups = []
    a = 0
    while a < nch:
        groups.append((a, min(a + gc, nch)))
        a += gc
    groups = tuple(groups)
    gNL = [sum(bl[a:b]) for a, b in groups]
    gNH = [sum(bh[a:b]) for a, b in groups]
    ioffL = np.concatenate([[0], np.cumsum([n * 8 for n in gNL])]).astype(int)
    ioffH = np.concatenate([[0], np.cumsum([n * 8 for n in gNH])]).astype(int)

    seg_starts = np.searchsorted(seg, np.arange(nseg + 1))
    pos_in_seg = np.arange(len(seg)) - seg_starts[seg]

    in_maps = []
    for r in range(R):
        rel_arr = np.zeros((P, btot), np.float32)
        w_arr = np.zeros((P, btot), np.float32)
        il_arr = np.zeros((P, max(int(ioffL[-1]), 1)), np.int16)
        ih_arr = np.zeros((P, max(int(ioffH[-1]), 1)), np.int16)

        for g, (c_lo, c_hi) in enumerate(groups):
            # index lists per group: concat chunks, LO then HI separately
            posL = 0
            posH = 0
            for ci in range(c_lo, c_hi):
                for reg, nb in ((0, bl[ci]), (1, bh[ci])):
                    if nb == 0:
                        continue
                    s0 = seg_starts[(r * nch + ci) * 2 + reg]
                    s1 = seg_starts[(r * nch + ci) * 2 + reg + 1]
                    k = s1 - s0
                    ids = np.zeros(nb * P, np.int16)
                    ids[:k] = idx_s[s0:s1]
                    rr = np.zeros(nb * P, np.float32)
                    rr[:k] = rel_s[s0:s1]
                    ww = np.zeros(nb * P, np.float32)
                    ww[:k] = w_s[s0:s1]
                    # rel/w: [P, nb] with edge i -> partition i%128, col i//128
                    cbase = int(coff[ci]) + (0 if reg == 0 else bl[ci])
                    rel_arr[:, cbase:cbase + nb] = rr.reshape(nb, P).T
                    w_arr[:, cbase:cbase + nb] = ww.reshape(nb, P).T
                    # idx: 16-wrap, replicated across 8 partition groups
                    cols = nb * 8
                    iwrap = ids.reshape(cols, 16).T  # [16, cols]
                    if reg == 0:
                        o = int(ioffL[g]) + posL
                        il_arr[:, o:o + cols] = np.tile(iwrap, (8, 1))
                        posL += cols
                    else:
                        o = int(ioffH[g]) + posH
                        ih_arr[:, o:o + cols] = np.tile(iwrap, (8, 1))
                        posH += cols

        lo = r * npc
        hi = min(N, lo + npc)
        ids_r = np.zeros(npc_pad, np.float32)
        ids_r[:hi - lo] = invd[lo:hi]
        ids_rc = np.ascontiguousarray(ids_r.reshape(nch, P).T)
        xres = np.zeros((npc_pad, D), np.float32)
        xres[:hi - lo] = x_pad[lo:hi]
        xres = xres.astype(ml_dtypes.bfloat16)

        m = {
            "xq": xq,
            "xres": xres,
            "w2": W2,
            "iota": iota,
            "rel": rel_arr,
            "wgt": w_arr,
            "idxlo": il_arr,
            "idxhi": ih_arr,
            "invdsct": ids_rc,
        }
        if include_c:
            qr = np.zeros(npc_pad, np.float32)
            qr[:hi - lo] = (qd * np.asarray(dst_scale, np.float32))[lo:hi]
            m["cb"] = cb
            m["qsct"] = np.ascontiguousarray(qr.reshape(nch, P).T)
        in_maps.append(m)

    geom = dict(nt=nt, bl=bl, bh=bh, groups=groups, include_c=include_c,
                npc=npc, npc_pad=npc_pad, N=N, R=R, nch=nch)
    return in_maps, geom


def fix_gather_queues(nc, gq=4):
    """Post-finalize: spread InstDMAGatherAnt across SWDGE queues, matching
    the tile scheduler's DMASW lane assignment (queue = lane % gq) so each
    DMA semaphore stays locked to one queue."""
    from concourse.tile_sem_assignment import PROC_NAME_TO_IDX
    lane_of = {PROC_NAME_TO_IDX[f"DMASW{i}"]: i for i in range(8)}
    for block in nc.m.functions[0].blocks:
        for inst in block.instructions:
            if type(inst).__name__ == "InstDMAGatherAnt":
                proc = getattr(inst, "bass_scheduled_proc", None)
                if proc in lane_of:
                    inst.queue_num = lane_of[proc] % gq


_PROGRAM_CACHE = {}


def kernel(x, gamma, beta, W, b, edge_index, num_nodes, edge_weight,
           dst_scale, n_cores=8, _collect=None):
    x = np.asarray(x)
    N = x.shape[0]
    in_maps, geom = prepare_inputs(
        np.asarray(x), np.asarray(gamma), np.asarray(beta), np.asarray(W),
        np.asarray(b), np.asarray(edge_index), np.asarray(edge_weight),
        np.asarray(dst_scale), n_cores)

    key = (geom["nt"], geom["bl"], geom["bh"], geom["groups"],
           geom["include_c"])
    nc = _PROGRAM_CACHE.get(key)
    if nc is None:
        nc = build_program(*key)
        nc.finalize()
        fix_gather_queues(nc)
        _PROGRAM_CACHE[key] = nc

    res = run_bass_kernel_spmd(nc, in_maps, list(range(n_cores)),
                               **(_collect.pop("kwargs") if _collect else {}))
    if _collect is not None:
        _collect["res"] = res

    y = np.empty((N, D), np.float32)
    npc = geom["npc"]
    for r in range(geom["R"]):
        lo = r * npc
        hi = min(N, lo + npc)
        y[lo:hi] = res.results[r]["y"][:hi - lo].astype(np.float32)
    return y


# revision 12
# speedup vs baseline: 7.3949x; 1.0314x over previous
"""Trainium2 Bass kernel for nn_DiWeightedGCNLayer (8-core SPMD), v3.

Math (per reference):
    h   = (x - mu) * rsqrt(var + eps)            # LN, gamma folded into W2
    m   = (h * gamma) @ W.T + b = h @ W2 (+ c)
    out = segsum(m[src] * w, dst) / max(deg, 1) * dst_scale
    y   = x + gelu(out)

Key structure (v3):
  - Matmul associativity: sum_e w_e * (h[src_e] @ W2) = (sum_e w_e *
    onehot_e (x) h[src_e]) @ W2.  Phase 1 only computes/stores h (LN) in
    bf16; phase 2 gathers h rows, accumulates U^T = sum_b msg_b^T @ oh_b
    per 128-dst chunk in PSUM, then applies W2 once per chunk.
  - Gathers use batched InstDMAGatherAnt (dma_gather): one instruction per
    (group-of-chunks, src-region) instead of one indirect DMA per 128
    edges (SWDGE fixed cost ~1us each was the old bottleneck).
  - dma_gather indices are int16: h is stored in two region tensors with
    tiled layout hq[p, t, :] = h[t*128 + p], so the flat row index
    (s%128)*ntile + s//128 stays < 32768 for both regions.
  - x input is pre-tiled/bf16 on the host (xq[p, t, :] = x[t*128+p]) so
    every load/store is 2KB-contiguous per partition.
  - deg never computed on device: host folds dst_scale/max(deg,1) into a
    per-node scalar.
Engine balance: DVE bn_stats/bn_aggr + one-hots; Pool normalize +
gathers; ACT rsqrt + h-stores + U copies + gelu; PE scatter matmuls; SP
x/meta/xres/y DMAs.
"""

import contextlib
import numpy as np
import ml_dtypes

import concourse.bass as bass
import concourse.bacc as bacc
import concourse.tile as tile
import concourse.mybir as mybir
from concourse.bass_utils import run_bass_kernel_spmd

F32 = mybir.dt.float32
BF16 = mybir.dt.bfloat16
I16 = mybir.dt.int16
AF = mybir.ActivationFunctionType
OP = mybir.AluOpType

D = 128
P = 128
LN_EPS = 1e-5
REG0_TILES = 256          # LO region = nodes [0, 256*128) = [0, 32768)
GT = 16                   # phase-1 tiles per x-load/h-store group


def build_program(nt, bl, bh, groups, include_c=False, af_gelu=True,
                  loop_n=1, gq=4, debug_phase=0):
    """One-core SPMD program.

    nt: total 128-row node tiles (incl. padding).
    bl/bh: per chunk-index LO/HI block counts (tuples, len nch).
    groups: tuple of (chunk_lo, chunk_hi) chunk-index ranges per gather
        group.
    """
    nch = len(bl)
    ntL = REG0_TILES
    ntH = nt - REG0_TILES
    btot = sum(bl) + sum(bh)
    # per-chunk column offset into rel/w arrays
    coff = np.concatenate([[0], np.cumsum(np.array(bl) + np.array(bh))])
    # per-group idx column offsets (int16 cols = n_idx/16 = blocks*8)
    gNL = [sum(bl[a:b]) for a, b in groups]
    gNH = [sum(bh[a:b]) for a, b in groups]
    ioffL = np.concatenate([[0], np.cumsum([n * 8 for n in gNL])])
    ioffH = np.concatenate([[0], np.cumsum([n * 8 for n in gNH])])

    nc = bacc.Bacc(num_swdge_queues=gq)

    xq_ext = nc.declare_dram_parameter("xq", [P, nt, D], BF16, isOutput=False)
    xres_ext = nc.declare_dram_parameter("xres", [nch * P, D], BF16,
                                         isOutput=False)
    w2_ext = nc.declare_dram_parameter("w2", [D, D], BF16, isOutput=False)
    iota_ext = nc.declare_dram_parameter("iota", [P, P], BF16, isOutput=False)
    rel_ext = nc.declare_dram_parameter("rel", [P, btot], F32, isOutput=False)
    wgt_ext = nc.declare_dram_parameter("wgt", [P, btot], F32, isOutput=False)
    il_ext = nc.declare_dram_parameter("idxlo", [P, max(int(ioffL[-1]), 1)],
                                       I16, isOutput=False)
    ih_ext = nc.declare_dram_parameter("idxhi", [P, max(int(ioffH[-1]), 1)],
                                       I16, isOutput=False)
    ids_ext = nc.declare_dram_parameter("invdsct", [P, nch], F32,
                                        isOutput=False)
    if include_c:
        cb_ext = nc.declare_dram_parameter("cb", [P, D], F32, isOutput=False)
        qs_ext = nc.declare_dram_parameter("qsct", [P, nch], F32,
                                           isOutput=False)
    y_ext = nc.declare_dram_parameter("y", [nch * P, D], BF16, isOutput=True)

    hlo_dram = nc.dram_tensor("h_lo", [P * ntL, D], BF16)
    hhi_dram = nc.dram_tensor("h_hi", [P * ntH, D], BF16)

    with tile.TileContext(nc) as tc:
        with (
            tc.tile_pool(name="const", bufs=1) as const,
            tc.tile_pool(name="xp", bufs=3) as xp,
            tc.tile_pool(name="hp", bufs=3) as hp,
            tc.tile_pool(name="stats", bufs=8) as sp,
            tc.tile_pool(name="small", bufs=12) as smp,
            tc.tile_pool(name="msgL", bufs=2) as msgLp,
            tc.tile_pool(name="msgH", bufs=2) as msgHp,
            tc.tile_pool(name="meta", bufs=3) as metp,
            tc.tile_pool(name="oh", bufs=100) as ohp,
            tc.tile_pool(name="ut", bufs=4) as utp,
            tc.tile_pool(name="ep", bufs=8) as epp,
            tc.tile_pool(name="ps_u", bufs=6, space="PSUM") as ps_u,
            tc.tile_pool(name="ps_o", bufs=2, space="PSUM") as ps_o,
        ):
            # --- constants ---
            w2_t = const.tile([D, D], BF16)
            nc.sync.dma_start(out=w2_t[:], in_=w2_ext[:, :])
            iota_t = const.tile([P, P], BF16)
            nc.sync.dma_start(out=iota_t[:], in_=iota_ext[:, :])
            eps_t = const.tile([P, 1], F32)
            nc.vector.memset(eps_t[:], LN_EPS)
            ids_t = const.tile([P, nch], F32)
            nc.sync.dma_start(out=ids_t[:], in_=ids_ext[:, :])
            cb_t = qs_t = None
            if include_c:
                cb_t = const.tile([P, D], F32)
                nc.sync.dma_start(out=cb_t[:], in_=cb_ext[:, :])
                qs_t = const.tile([P, nch], F32)
                nc.sync.dma_start(out=qs_t[:], in_=qs_ext[:, :])

            loop_ctx = (tc.For_i(0, loop_n, 1) if loop_n > 1
                        else contextlib.nullcontext())
            with loop_ctx:
                # --- phase 1: h = LN(x), bf16, tiled layout, to HBM ---
                for t0 in range(0, nt, GT):
                    g_n = min(GT, nt - t0)
                    x8 = xp.tile([P, GT, D], BF16)
                    nc.sync.dma_start(out=x8[:, :g_n, :],
                                      in_=xq_ext[:, t0:t0 + g_n, :])
                    # grouped bn_stats: [P, 4, 128] -> [P, 4, 6]
                    # 6-tuple = (n_e, mean_e, n_e*var_e, n_o, mean_o,
                    # n_o*var_o) over even/odd elements (64 each); combine:
                    # mean = (m_e+m_o)/2, var = (s2_e+s2_o)/128 + (m_e-m_o)^2/4
                    # walrus rejects multi-group BNStats (out must be 6
                    # elems/partition) -> one bn_stats per tile, shared tile
                    st8 = sp.tile([P, GT, 6], F32, tag="st")
                    for j in range(g_n):
                        nc.vector.bn_stats(out=st8[:, j, :],
                                           in_=x8[:, j, :])
                    me = st8[:, :g_n, 1]
                    mo = st8[:, :g_n, 4]
                    s8 = smp.tile([P, GT], F32, tag="s8")
                    nc.vector.tensor_tensor(out=s8[:, :g_n], in0=me, in1=mo,
                                            op=OP.add)
                    d8 = smp.tile([P, GT], F32, tag="d8")
                    nc.vector.tensor_tensor(out=d8[:, :g_n], in0=me, in1=mo,
                                            op=OP.subtract)
                    v8 = smp.tile([P, GT], F32, tag="v8")
                    nc.vector.tensor_tensor(out=v8[:, :g_n],
                                            in0=st8[:, :g_n, 2],
                                            in1=st8[:, :g_n, 5], op=OP.add)
                    u8 = smp.tile([P, GT], F32, tag="u8")
                    nc.vector.scalar_tensor_tensor(
                        out=u8[:, :g_n], in0=d8[:, :g_n], scalar=0.25,
                        in1=d8[:, :g_n], op0=OP.mult, op1=OP.mult)
                    var8 = smp.tile([P, GT], F32, tag="var8")
                    nc.vector.scalar_tensor_tensor(
                        out=var8[:, :g_n], in0=v8[:, :g_n], scalar=1.0 / 128,
                        in1=u8[:, :g_n], op0=OP.mult, op1=OP.add)
                    sd8 = smp.tile([P, GT], F32, tag="sd")
                    nc.scalar.activation(out=sd8[:, :g_n], in_=var8[:, :g_n],
                                         func=AF.Sqrt, bias=eps_t[:, :],
                                         scale=1.0)
                    rstd8 = smp.tile([P, GT], F32, tag="rstd")
                    nc.vector.reciprocal(out=rstd8[:, :g_n],
                                         in_=sd8[:, :g_n])
                    bias8 = smp.tile([P, GT], F32, tag="bias")
                    nc.vector.scalar_tensor_tensor(
                        out=bias8[:, :g_n], in0=s8[:, :g_n], scalar=-0.5,
                        in1=rstd8[:, :g_n], op0=OP.mult, op1=OP.mult)
                    h8 = hp.tile([P, GT, D], BF16)
                    for j in range(g_n):
                        nc.vector.tensor_scalar(out=h8[:, j, :],
                                                in0=x8[:, j, :],
                                                scalar1=rstd8[:, j:j + 1],
                                                scalar2=bias8[:, j:j + 1],
                                                op0=OP.mult, op1=OP.add)
                    if t0 < ntL:
                        h_dst = hlo_dram.rearrange("(p t) d -> p t d", p=P)
                        nc.scalar.dma_start(out=h_dst[:, t0:t0 + g_n, :],
                                            in_=h8[:, :g_n, :])
                    else:
                        h_dst = hhi_dram.rearrange("(p t) d -> p t d", p=P)
                        nc.scalar.dma_start(
                            out=h_dst[:, t0 - ntL:t0 - ntL + g_n, :],
                            in_=h8[:, :g_n, :])

                # --- phase 2: gather + scatter-matmul per chunk ---
                if debug_phase == 1:
                    for ci in range(nch):
                        xr = epp.tile([P, D], BF16, tag="xr")
                        nc.sync.dma_start(
                            out=xr[:],
                            in_=xres_ext[ci * P:(ci + 1) * P, :])
                        nc.sync.dma_start(out=y_ext[ci * P:(ci + 1) * P, :],
                                          in_=xr[:])
                for g, (c_lo, c_hi) in enumerate(groups):
                    if debug_phase == 1:
                        break
                    nbl = gNL[g]
                    nbh = gNH[g]
                    bt_g = int(coff[c_hi] - coff[c_lo])
                    rl = metp.tile([P, bt_g], F32, tag="rel")
                    nc.sync.dma_start(out=rl[:],
                                      in_=rel_ext[:, coff[c_lo]:coff[c_hi]])
                    wg = metp.tile([P, bt_g], F32, tag="wgt")
                    nc.sync.dma_start(out=wg[:],
                                      in_=wgt_ext[:, coff[c_lo]:coff[c_hi]])
                    msgL = msgHq = None
                    if nbl:
                        ilt = metp.tile([P, nbl * 8], I16, tag="il")
                        nc.sync.dma_start(
                            out=ilt[:],
                            in_=il_ext[:, ioffL[g]:ioffL[g + 1]])
                        msgL = msgLp.tile([P, nbl, D], BF16)
                        qn = max(1, nbl // 4)
                        b0 = 0
                        while b0 < nbl:
                            b1 = min(b0 + qn, nbl)
                            nc.gpsimd.dma_gather(
                                msgL[:, b0:b1, :], hlo_dram[:, :],
                                ilt[:, b0 * 8:b1 * 8],
                                (b1 - b0) * 128, (b1 - b0) * 128, D,
                                queue_num=0, single_packet=False)
                            b0 = b1
                    if nbh:
                        iht = metp.tile([P, nbh * 8], I16, tag="ih")
                        nc.sync.dma_start(
                            out=iht[:],
                            in_=ih_ext[:, ioffH[g]:ioffH[g + 1]])
                        msgHq = msgHp.tile([P, nbh, D], BF16)
                        qh = max(1, nbh // 2)
                        b0 = 0
                        while b0 < nbh:
                            b1 = min(b0 + qh, nbh)
                            nc.gpsimd.dma_gather(
                                msgHq[:, b0:b1, :], hhi_dram[:, :],
                                iht[:, b0 * 8:b1 * 8],
                                (b1 - b0) * 128, (b1 - b0) * 128, D,
                                queue_num=0, single_packet=False)
                            b0 = b1

                    n_groups = len(groups)
                    oh_pool_g = False
                    blkL = 0
                    blkH = 0
                    if debug_phase == 2:
                        for ci in range(c_lo, c_hi):
                            xr = epp.tile([P, D], BF16, tag="xr")
                            src_t = msgL if msgL is not None else msgHq
                            nc.vector.tensor_copy(out=xr[:],
                                                  in_=src_t[:, 0, :])
                            nc.sync.dma_start(
                                out=y_ext[ci * P:(ci + 1) * P, :], in_=xr[:])
                        continue
                    for ci in range(c_lo, c_hi):
                        nL, nH = bl[ci], bh[ci]
                        nb_c = nL + nH
                        if nb_c == 0:
                            continue
                        col0 = int(coff[ci] - coff[c_lo])
                        ut_ps = ps_u.tile([P, D], F32)
                        for k in range(nb_c):
                            col = col0 + k
                            oh = ohp.tile([P, P], BF16)
                            oh_eng = (nc.gpsimd
                                      if (oh_pool_g and col % 2 == 0)
                                      else nc.vector)
                            oh_eng.tensor_scalar(out=oh[:], in0=iota_t[:],
                                                scalar1=rl[:, col:col + 1],
                                                scalar2=wg[:, col:col + 1],
                                                op0=OP.is_equal,
                                                op1=OP.mult)
                            msrc = (msgL[:, blkL + k, :] if k < nL
                                    else msgHq[:, blkH + k - nL, :])
                            nc.tensor.matmul(out=ut_ps[:], lhsT=msrc,
                                             rhs=oh[:], start=(k == 0),
                                             stop=(k == nb_c - 1))
                        blkL += nL
                        blkH += nH

                        ut_sb = utp.tile([P, D], BF16)
                        nc.scalar.copy(out=ut_sb[:], in_=ut_ps[:])
                        o2_ps = ps_o.tile([P, D], F32)
                        nc.tensor.matmul(out=o2_ps[:], lhsT=ut_sb[:],
                                         rhs=w2_t[:], start=True, stop=True)
                        sc = epp.tile([P, D], F32, tag="sc")
                        nc.vector.tensor_scalar(out=sc[:], in0=o2_ps[:],
                                                scalar1=ids_t[:, ci:ci + 1],
                                                scalar2=None, op0=OP.mult)
                        if include_c:
                            cc = epp.tile([P, D], F32, tag="cc")
                            nc.vector.tensor_scalar(out=cc[:], in0=cb_t[:],
                                                    scalar1=qs_t[:, ci:ci + 1],
                                                    scalar2=None, op0=OP.mult)
                            nc.vector.tensor_tensor(out=sc[:], in0=sc[:],
                                                    in1=cc[:], op=OP.add)
                        gl = epp.tile([P, D], F32, tag="gl")
                        if af_gelu == "tanh1":
                            nc.scalar.activation(out=gl[:], in_=sc[:],
                                                 func=AF.Tanh)
                        elif af_gelu:
                            nc.scalar.activation(out=gl[:], in_=sc[:],
                                                 func=AF.Gelu)
                        else:
                            # tanh-gelu composition (CoreSim lacks Gelu table)
                            sq = epp.tile([P, D], F32, tag="sq")
                            nc.vector.tensor_mul(out=sq[:], in0=sc[:],
                                                 in1=sc[:])
                            cu = epp.tile([P, D], F32, tag="cu")
                            nc.vector.tensor_mul(out=cu[:], in0=sq[:],
                                                 in1=sc[:])
                            u = epp.tile([P, D], F32, tag="u")
                            nc.vector.tensor_scalar(out=u[:], in0=cu[:],
                                                    scalar1=0.044715,
                                                    scalar2=None, op0=OP.mult)
                            nc.vector.tensor_add(out=u[:], in0=u[:], in1=sc[:])
                            v = epp.tile([P, D], F32, tag="v")
                            nc.scalar.activation(out=v[:], in_=u[:],
                                                 func=AF.Tanh,
                                                 scale=0.7978845608028654)
                            w1 = epp.tile([P, D], F32, tag="w1")
                            nc.vector.tensor_mul(out=w1[:], in0=sc[:], in1=v[:])
                            nc.vector.tensor_add(out=w1[:], in0=w1[:],
                                                 in1=sc[:])
                            nc.vector.tensor_scalar(out=gl[:], in0=w1[:],
                                                    scalar1=0.5, scalar2=None,
                                                    op0=OP.mult)
                        xr = epp.tile([P, D], BF16, tag="xr")
                        nc.sync.dma_start(
                            out=xr[:],
                            in_=xres_ext[ci * P:(ci + 1) * P, :])
                        yt = epp.tile([P, D], BF16, tag="yt")
                        nc.vector.tensor_tensor(out=yt[:], in0=gl[:],
                                                in1=xr[:], op=OP.add)
                        nc.sync.dma_start(out=y_ext[ci * P:(ci + 1) * P, :],
                                          in_=yt[:])

    return nc


def prepare_inputs(x, gamma, beta, W, b, edge_index, edge_weight, dst_scale,
                   n_cores, gc=7):
    """Host-side prep: tile/cast x, fold params, sort edges by (core,
    chunk, region, src), build gather indices + one-hot metadata."""
    N = x.shape[0]
    R = n_cores
    npc = (N + R - 1) // R
    nch = (npc + P - 1) // P
    npc_pad = nch * P
    n_pad = (((R - 1) * npc + npc_pad + P - 1) // P) * P
    nt = n_pad // P
    ntL = REG0_TILES
    ntH = nt - ntL
    NLO = ntL * P

    x = np.asarray(x, np.float32)
    x_pad = np.zeros((n_pad, D), np.float32)
    x_pad[:N] = x
    # tiled bf16 layout xq[p, t, :] = x[t*128 + p]
    xq = np.ascontiguousarray(
        x_pad.reshape(nt, P, D).transpose(1, 0, 2)).astype(ml_dtypes.bfloat16)

    W2 = (np.asarray(W, np.float32).T * np.asarray(gamma, np.float32)[:, None])
    W2 = np.ascontiguousarray(W2).astype(ml_dtypes.bfloat16)
    c = (np.asarray(beta, np.float32) @ np.asarray(W, np.float32).T
         + np.asarray(b, np.float32))
    include_c = bool(np.any(c != 0.0))
    cb = np.ascontiguousarray(np.broadcast_to(c, (P, D))).astype(np.float32)

    iota = np.broadcast_to(np.arange(P, dtype=np.float32), (P, P))
    iota = np.ascontiguousarray(iota).astype(ml_dtypes.bfloat16)

    src = np.asarray(edge_index[0]).astype(np.int64)
    dst = np.asarray(edge_index[1]).astype(np.int64)
    w = np.asarray(edge_weight, np.float32)

    deg = np.bincount(dst, weights=w.astype(np.float64),
                      minlength=N).astype(np.float32)
    dmax = np.maximum(deg, 1.0)
    invd = np.asarray(dst_scale, np.float32) / dmax
    qd = deg / dmax  # deg/max(deg,1): multiplier for the c term

    core_id = np.minimum(dst // npc, R - 1)
    local = dst - core_id * npc
    chunk_id = local // P
    rel = (local % P).astype(np.float32)
    region = (src >= NLO).astype(np.int64)
    # tiled flat row index within region
    idx = np.where(region == 0,
                   (src % P) * ntL + src // P,
                   (src % P) * ntH + (src - NLO) // P).astype(np.int64)

    # sort by (core, chunk, region, src) for gather locality
    key = ((core_id * nch + chunk_id) * 2 + region) * (1 << 17) + src
    order = np.argsort(key, kind="stable")
    idx_s = idx[order]
    rel_s = rel[order]
    w_s = w[order]
    seg = ((core_id * nch + chunk_id) * 2 + region)[order]

    nseg = R * nch * 2
    cnt = np.bincount(seg, minlength=nseg).reshape(R, nch, 2)
    blocks = -(-cnt // P)  # ceil
    bl = blocks[:, :, 0].max(axis=0)
    bh = blocks[:, :, 1].max(axis=0)
    # every chunk needs >= 1 block total for PSUM start/stop
    empty = (bl + bh) == 0
    bl[empty] = 1
    bl = tuple(int(v) for v in bl)
    bh = tuple(int(v) for v in bh)
    btot = sum(bl) + sum(bh)
    coff = np.concatenate([[0], np.cumsum(np.array(bl) + np.array(bh))])

    groups = []
    a = 0
    while a < nch:
        groups.append((a, min(a + gc, nch)))
        a += gc
    groups = tuple(groups)
    gNL = [sum(bl[a:b]) for a, b in groups]
    gNH = [sum(bh[a:b]) for a, b in groups]
    ioffL = np.concatenate([[0], np.cumsum([n * 8 for n in gNL])]).astype(int)
    ioffH = np.concatenate([[0], np.cumsum([n * 8 for n in gNH])]).astype(int)

    seg_starts = np.searchsorted(seg, np.arange(nseg + 1))
    pos_in_seg = np.arange(len(seg)) - seg_starts[seg]

    in_maps = []
    for r in range(R):
        rel_arr = np.zeros((P, btot), np.float32)
        w_arr = np.zeros((P, btot), np.float32)
        il_arr = np.zeros((P, max(int(ioffL[-1]), 1)), np.int16)
        ih_arr = np.zeros((P, max(int(ioffH[-1]), 1)), np.int16)

        for g, (c_lo, c_hi) in enumerate(groups):
            # index lists per group: concat chunks, LO then HI separately
            posL = 0
            posH = 0
            for ci in range(c_lo, c_hi):
                for reg, nb in ((0, bl[ci]), (1, bh[ci])):
                    if nb == 0:
                        continue
                    s0 = seg_starts[(r * nch + ci) * 2 + reg]
                    s1 = seg_starts[(r * nch + ci) * 2 + reg + 1]
                    k = s1 - s0
                    ids = np.zeros(nb * P, np.int16)
                    ids[:k] = idx_s[s0:s1]
                    rr = np.zeros(nb * P, np.float32)
                    rr[:k] = rel_s[s0:s1]
                    ww = np.zeros(nb * P, np.float32)
                    ww[:k] = w_s[s0:s1]
                    # rel/w: [P, nb] with edge i -> partition i%128, col i//128
                    cbase = int(coff[ci]) + (0 if reg == 0 else bl[ci])
                    rel_arr[:, cbase:cbase + nb] = rr.reshape(nb, P).T
                    w_arr[:, cbase:cbase + nb] = ww.reshape(nb, P).T
                    # idx: 16-wrap, replicated across 8 partition groups
                    cols = nb * 8
                    iwrap = ids.reshape(cols, 16).T  # [16, cols]
                    if reg == 0:
                        o = int(ioffL[g]) + posL
                        il_arr[:, o:o + cols] = np.tile(iwrap, (8, 1))
                        posL += cols
                    else:
                        o = int(ioffH[g]) + posH
                        ih_arr[:, o:o + cols] = np.tile(iwrap, (8, 1))
                        posH += cols

        lo = r * npc
        hi = min(N, lo + npc)
        ids_r = np.zeros(npc_pad, np.float32)
        ids_r[:hi - lo] = invd[lo:hi]
        ids_rc = np.ascontiguousarray(ids_r.reshape(nch, P).T)
        xres = np.zeros((npc_pad, D), np.float32)
        xres[:hi - lo] = x_pad[lo:hi]
        xres = xres.astype(ml_dtypes.bfloat16)

        m = {
            "xq": xq,
            "xres": xres,
            "w2": W2,
            "iota": iota,
            "rel": rel_arr,
            "wgt": w_arr,
            "idxlo": il_arr,
            "idxhi": ih_arr,
            "invdsct": ids_rc,
        }
        if include_c:
            qr = np.zeros(npc_pad, np.float32)
            qr[:hi - lo] = (qd * np.asarray(dst_scale, np.float32))[lo:hi]
            m["cb"] = cb
            m["qsct"] = np.ascontiguousarray(qr.reshape(nch, P).T)
        in_maps.append(m)

    geom = dict(nt=nt, bl=bl, bh=bh, groups=groups, include_c=include_c,
                npc=npc, npc_pad=npc_pad, N=N, R=R, nch=nch)
    return in_maps, geom


def fix_gather_queues(nc, gq=4):
    """Post-finalize: spread InstDMAGatherAnt across SWDGE queues, matching
    the tile scheduler's DMASW lane assignment (queue = lane % gq) so each
    DMA semaphore stays locked to one queue."""
    from concourse.tile_sem_assignment import PROC_NAME_TO_IDX
    lane_of = {PROC_NAME_TO_IDX[f"DMASW{i}"]: i for i in range(8)}
    for block in nc.m.functions[0].blocks:
        for inst in block.instructions:
            if type(inst).__name__ == "InstDMAGatherAnt":
                proc = getattr(inst, "bass_scheduled_proc", None)
                if proc in lane_of:
                    inst.queue_num = lane_of[proc] % gq


_PROGRAM_CACHE = {}


def kernel(x, gamma, beta, W, b, edge_index, num_nodes, edge_weight,
           dst_scale, n_cores=8, _collect=None):
    x = np.asarray(x)
    N = x.shape[0]
    in_maps, geom = prepare_inputs(
        np.asarray(x), np.asarray(gamma), np.asarray(beta), np.asarray(W),
        np.asarray(b), np.asarray(edge_index), np.asarray(edge_weight),
        np.asarray(dst_scale), n_cores)

    key = (geom["nt"], geom["bl"], geom["bh"], geom["groups"],
           geom["include_c"])
    nc = _PROGRAM_CACHE.get(key)
    if nc is None:
        nc = build_program(*key)
        nc.finalize()
        fix_gather_queues(nc)
        _PROGRAM_CACHE[key] = nc

    res = run_bass_kernel_spmd(nc, in_maps, list(range(n_cores)),
                               **(_collect.pop("kwargs") if _collect else {}))
    if _collect is not None:
        _collect["res"] = res

    y = np.empty((N, D), np.float32)
    npc = geom["npc"]
    for r in range(geom["R"]):
        lo = r * npc
        hi = min(N, lo + npc)
        y[lo:hi] = res.results[r]["y"][:hi - lo].astype(np.float32)
    return y
